# revision 7
# baseline (speedup 1.0000x reference)
"""Trainium2 Bass kernel for nn_GAT_86045374808682 (3-layer GAT + coordinate head).

Self-contained: takes FULL inputs, shards across 8 NeuronCores internally,
returns the FULL [8192, 2] float32 output.

Strategy:
- Nodes relabeled by in-degree desc; 64 blocks of 128 striped across 8 cores
  (block j -> core j%8), so every core sees the same per-stripe padded degree
  schedule K[t] (SPMD: one program, identical shapes on all cores).
- Per-layer node table T[v] = [h(128) | sa(8) | da(8) | pad(48)] f32 (768B rows),
  row-gathered per edge-slot with gpsimd.dma_gather (dst-lane on partition,
  slots along free dim, slot-major index lists built on host).
- Edge phase per stripe, chunked by 16 slots: gather -> scores (narrow per-head)
  -> ex=exp(leaky_relu) -> w = h_g*ex_rep (DVE) -> PE transpose-accumulate over
  slots into PSUM -> divide by den (PE-replicated reciprocal) at stripe end.
- LN/ReLU in feature-major via PE ones-matmuls; rstd = exp(-0.5 ln(var+eps))
  with one Newton polish; tanh/softplus composed from exp/ln (single ACT table).
- 5 launches, 4 programs: P1 (x@W1 fp32), P2 x2 (edge+node+pack), P3 (edge+MLP
  head -> angles/radius), P4 (trig finalize, replicated). Host concats slabs.
"""
import os
import sys

import numpy as np

for _p in ("/opt/trn_rl_repo", "/root/.axon_site/_ro/trn_rl_repo"):
    if _p not in sys.path:
        sys.path.append(_p)

import concourse.bass as bass  # noqa: F401
import concourse.tile as tile
from concourse import bacc, library_config, mybir
from concourse.masks import make_identity

dt = mybir.dt
AF = mybir.ActivationFunctionType
OP = mybir.AluOpType

N = 8192
IN = 8193
INP = 8320  # 65 * 128
H = 8
HC = 128
P = 128
NCORES = 8
NSTRIPE = 8
KC = 16  # gather chunk (slots)
MASKVAL = -1e5
PI = float(np.pi)


# ----------------------------------------------------------------------------
# host-side graph prep
# ----------------------------------------------------------------------------

def host_prep(src, dst):
    s = np.concatenate([np.asarray(src).astype(np.int64), np.arange(N, dtype=np.int64)])
    d = np.concatenate([np.asarray(dst).astype(np.int64), np.arange(N, dtype=np.int64)])
    deg = np.bincount(d, minlength=N)
    order = np.argsort(-deg, kind="stable")  # new-id -> old-id
    old2new = np.empty(N, np.int64)
    old2new[order] = np.arange(N)
    s_new = old2new[s]
    d_new = old2new[d]
    deg_new = deg[order]

    K = [int(deg_new[1024 * t]) for t in range(NSTRIPE)]  # desc-sorted -> stripe max
    offs = np.cumsum([0] + K)

    eo = np.argsort(d_new, kind="stable")
    s_sorted = s_new[eo]
    starts = np.searchsorted(d_new[eo], np.arange(N))

    idxq = np.zeros((NCORES, 16, int(offs[-1]) * 8), np.int16)
    maskq = np.full((NCORES, P, int(offs[-1])), MASKVAL, np.float32)
    ar = np.arange(P)
    for c in range(NCORES):
        for t in range(NSTRIPE):
            Kt = K[t]
            vids = (t * NCORES + c) * P + ar
            e0 = starts[vids]
            degs = deg_new[vids]
            kk = np.arange(Kt)
            take = np.minimum(e0[:, None] + kk[None, :], len(s_sorted) - 1)
            mat = s_sorted[take]                      # [128, Kt]
            valid = kk[None, :] < degs[:, None]
            mat = np.where(valid, mat, 0)
            maskq[c, :, offs[t] : offs[t] + Kt] = np.where(valid, 0.0, MASKVAL)
            lin = mat.T.reshape(-1)                   # slot-major [Kt*128]
            o16 = int(offs[t]) * 8
            idxq[c, :, o16 : o16 + Kt * 8] = lin.reshape(-1, 16).T
    return dict(order=order, K=K, offs=offs, idxq=idxq.astype(np.int16), maskq=maskq)


def core_cols(c):
    return np.concatenate([np.arange((t * NCORES + c) * P, (t * NCORES + c) * P + P)
                           for t in range(NSTRIPE)])


def mboth(a_src, a_dst):
    M = np.zeros((P, 16), np.float32)
    for h in range(H):
        M[h * 16 : (h + 1) * 16, h] = a_src[h]
        M[h * 16 : (h + 1) * 16, 8 + h] = a_dst[h]
    return M


# ----------------------------------------------------------------------------
# shared bass building blocks
# ----------------------------------------------------------------------------

def _mk_consts(nc, consts):
    c = {"pool": consts}
    nc.gpsimd.load_library(library_config.mlp)
    c["ident"] = consts.tile([P, P], dt.float32, name="c_ident")
    make_identity(nc, c["ident"][:])
    c["ones_col"] = consts.tile([P, 1], dt.float32, name="c_ones_col")
    nc.gpsimd.memset(c["ones_col"][:], 1.0)
    c["ones_row"] = consts.tile([1, P], dt.float32, name="c_ones_row")
    nc.gpsimd.memset(c["ones_row"][:], 1.0)
    c["eps"] = consts.tile([1, 1], dt.float32, name="c_eps")
    nc.gpsimd.memset(c["eps"][:], 1e-5)
    return c


def _rstd(nc, sb, var_ap, out_ap, n, eps):
    """out = 1/sqrt(var + eps): exp(-0.5 ln(var+eps)) + one Newton polish."""
    if eps:
        vpe = sb.tile([1, 512], dt.float32, tag="rs_vpe")
        nc.vector.tensor_scalar_add(vpe[:, 0:n], var_ap, float(eps))
        var_ap = vpe[:, 0:n]
    lnv = sb.tile([1, 512], dt.float32, tag="rs_ln")
    nc.scalar.activation(out=lnv[:, 0:n], in_=var_ap, func=AF.Ln)
    y = sb.tile([1, 512], dt.float32, tag="rs_y")
    nc.scalar.activation(out=y[:, 0:n], in_=lnv[:, 0:n], func=AF.Exp, scale=-0.5)
    u = sb.tile([1, 512], dt.float32, tag="rs_u")
    nc.vector.tensor_tensor(out=u[:, 0:n], in0=y[:, 0:n], in1=y[:, 0:n], op=OP.mult)
    nc.vector.tensor_tensor(out=u[:, 0:n], in0=u[:, 0:n], in1=var_ap, op=OP.mult)
    nc.vector.tensor_scalar(out=u[:, 0:n], in0=u[:, 0:n], scalar1=-0.5, scalar2=1.5,
                            op0=OP.mult, op1=OP.add)
    nc.vector.tensor_tensor(out=out_ap, in0=y[:, 0:n], in1=u[:, 0:n], op=OP.mult)


def _ln_relu_fm(nc, sb, ps, c, x_sb, n, gamma_t, beta_t, out_sb, nfeat=P):
    """Feature-major LN + affine + ReLU: out = relu(gamma*(x-mu)*rstd + beta).
    x_sb [nfeat, n] SBUF; per-column stats; processed in 512-col chunks."""
    for j in range(0, n, 512):
        w = min(512, n - j)
        xs = x_sb[:, j : j + w]
        xsq = sb.tile([nfeat, 512], dt.float32, tag="ln_xsq")
        nc.scalar.activation(out=xsq[:, 0:w], in_=xs, func=AF.Square)
        s1_ps = ps.tile([1, 512], dt.float32, space="PSUM", tag="pp_a")
        nc.tensor.matmul(out=s1_ps[:, 0:w], lhsT=c["ones_col"][0:nfeat, :], rhs=xs,
                         start=True, stop=True)
        s2_ps = ps.tile([1, 512], dt.float32, space="PSUM", tag="pp_b")
        nc.tensor.matmul(out=s2_ps[:, 0:w], lhsT=c["ones_col"][0:nfeat, :],
                         rhs=xsq[:, 0:w], start=True, stop=True)
        mu = sb.tile([1, 512], dt.float32, tag="ln_mu")
        nc.vector.tensor_scalar_mul(mu[:, 0:w], s1_ps[:, 0:w], 1.0 / nfeat)
        musq = sb.tile([1, 512], dt.float32, tag="ln_musq")
        nc.scalar.activation(out=musq[:, 0:w], in_=mu[:, 0:w], func=AF.Square)
        var = sb.tile([1, 512], dt.float32, tag="ln_var")
        nc.vector.scalar_tensor_tensor(out=var[:, 0:w], in0=s2_ps[:, 0:w],
                                       scalar=1.0 / nfeat, in1=musq[:, 0:w],
                                       op0=OP.mult, op1=OP.subtract)
        rs = sb.tile([1, 512], dt.float32, tag="ln_rs")
        _rstd(nc, sb, var[:, 0:w], rs[:, 0:w], w, 1e-5)
        rep_mu = ps.tile([nfeat, 512], dt.float32, space="PSUM", tag="pp_a")
        nc.tensor.matmul(out=rep_mu[:, 0:w], lhsT=c["ones_row"][:, 0:nfeat],
                         rhs=mu[:, 0:w], start=True, stop=True)
        rep_rs = ps.tile([nfeat, 512], dt.float32, space="PSUM", tag="pp_b")
        nc.tensor.matmul(out=rep_rs[:, 0:w], lhsT=c["ones_row"][:, 0:nfeat],
                         rhs=rs[:, 0:w], start=True, stop=True)
        xh = sb.tile([nfeat, 512], dt.float32, tag="ln_xh")
        nc.vector.tensor_tensor(out=xh[:, 0:w], in0=xs, in1=rep_mu[:, 0:w], op=OP.subtract)
        nc.vector.tensor_tensor(out=xh[:, 0:w], in0=xh[:, 0:w], in1=rep_rs[:, 0:w],
                                op=OP.mult)
        nc.scalar.activation(out=out_sb[:, j : j + w], in_=xh[:, 0:w], func=AF.Relu,
                             scale=gamma_t[:], bias=beta_t[:])


def _edge_stripe(nc, c, sb, gpool, wpool, ps, psagg, Tfull, idx_t, mask_t, da_stripe,
                 K_t, off_t, agg_sb, rep16_t):
    """One stripe: gather + segment softmax + weighted sum for 128 dst lanes.
    Writes normalized aggregation (feature-major [128 f, 128 dst]) to agg_sb."""
    nchunk = (K_t + KC - 1) // KC
    agg = psagg.tile([P, P], dt.float32, space="PSUM", tag="agg")
    den = sb.tile([P, 8], dt.float32, tag="den")
    for ci in range(nchunk):
        k0 = ci * KC
        kc = min(KC, K_t - k0)
        g = gpool.tile([P, KC, 192], dt.float32, tag="gather")
        nc.gpsimd.dma_gather(
            out_ap=g[:, 0:kc, :],
            in_ap=Tfull[:],
            idxs_ap=idx_t[:, (off_t + k0) * 8 : (off_t + k0 + kc) * 8],
            num_idxs=kc * P,
            num_idxs_reg=kc * P,
            elem_size=192,
            single_packet=False,
        )
        z = sb.tile([P, KC, 8], dt.float32, tag="z")
        nc.vector.tensor_tensor(out=z[:, 0:kc, :], in0=g[:, 0:kc, 128:136],
                                in1=da_stripe.unsqueeze(1).to_broadcast([P, kc, 8]),
                                op=OP.add)
        nc.vector.tensor_tensor(
            out=z[:, 0:kc, :], in0=z[:, 0:kc, :],
            in1=mask_t[:, off_t + k0 : off_t + k0 + kc].unsqueeze(2).to_broadcast([P, kc, 8]),
            op=OP.add)
        zl = sb.tile([P, KC, 8], dt.float32, tag="zl")
        nc.vector.tensor_scalar_mul(zl[:, 0:kc, :], z[:, 0:kc, :], 0.2)
        nc.vector.tensor_tensor(out=zl[:, 0:kc, :], in0=zl[:, 0:kc, :], in1=z[:, 0:kc, :],
                                op=OP.max)
        ex = sb.tile([P, KC, 8], dt.float32, tag="ex")
        nc.scalar.activation(out=ex[:, 0:kc, :], in_=zl[:, 0:kc, :], func=AF.Exp)
        dc = sb.tile([P, 8], dt.float32, tag="dc")
        nc.vector.tensor_reduce(out=dc[:], in_=ex[:, 0:kc, :].transpose([0, 2, 1]),
                                axis=mybir.AxisListType.X, op=OP.add)
        if ci == 0:
            nc.vector.tensor_copy(out=den[:], in_=dc[:])
        else:
            nc.vector.tensor_tensor(out=den[:], in0=den[:], in1=dc[:], op=OP.add)
        w = wpool.tile([P, KC, P], dt.float32, tag="w")
        nc.vector.tensor_tensor(
            out=w[:, 0:kc, :].rearrange("p k (h e) -> p k h e", h=8),
            in0=g[:, 0:kc, 0:128].rearrange("p k (h e) -> p k h e", h=8),
            in1=ex[:, 0:kc, :].unsqueeze(3).to_broadcast([P, kc, 8, 16]),
            op=OP.mult)
        for k in range(kc):
            nc.tensor.matmul(out=agg[:], lhsT=w[:, k, :], rhs=c["ident"][:],
                             is_transpose=True, start=(ci == 0 and k == 0),
                             stop=(ci == nchunk - 1 and k == kc - 1))
    dent = ps.tile([8, P], dt.float32, space="PSUM", tag="pp_a")
    nc.tensor.matmul(out=dent[0:8, :], lhsT=den[:], rhs=c["ident"][:],
                     is_transpose=True, start=True, stop=True)
    rden = sb.tile([8, P], dt.float32, tag="rden")
    nc.vector.reciprocal(out=rden[:], in_=dent[0:8, :])
    rdrep = ps.tile([P, P], dt.float32, space="PSUM", tag="pp_b")
    nc.tensor.matmul(out=rdrep[:], lhsT=rep16_t[:], rhs=rden[:], start=True, stop=True)
    rdrep_sb = sb.tile([P, P], dt.float32, tag="rdrep_sb")
    nc.vector.tensor_copy(out=rdrep_sb[:], in_=rdrep[:])
    nc.vector.tensor_tensor(out=agg_sb, in0=agg[:], in1=rdrep_sb[:], op=OP.mult)


# ----------------------------------------------------------------------------
# program builders
# ----------------------------------------------------------------------------

def build_p1():
    nc = bacc.Bacc(None, target_bir_lowering=False)
    xT = nc.declare_dram_parameter("xT", [INP, 1024], dt.float32, isOutput=False)
    W1 = nc.declare_dram_parameter("W1", [INP, HC], dt.float32, isOutput=False)
    Mb = nc.declare_dram_parameter("Mb", [P, 16], dt.float32, isOutput=False)
    Tout = nc.declare_dram_parameter("Tout", [1024, 192], dt.float32, isOutput=True)

    with tile.TileContext(nc) as tc:
        with (
            tc.tile_pool(name="consts", bufs=1) as consts,
            tc.tile_pool(name="wpool", bufs=3) as wp,
            tc.tile_pool(name="xpool", bufs=3) as xp,
            tc.tile_pool(name="sb", bufs=2) as sb,
            tc.tile_pool(name="psh", bufs=2, space="PSUM") as psh,
            tc.tile_pool(name="ps", bufs=2, space="PSUM") as ps,
        ):
            ident = consts.tile([P, P], dt.float32)
            make_identity(nc, ident[:])
            mb_t = consts.tile([P, 16], dt.float32)
            nc.sync.dma_start(out=mb_t[:], in_=Mb[:])
            for nb in range(2):
                hps = psh.tile([P, 512], dt.float32, space="PSUM", tag="hps")
                for kcb in range(65):
                    wt = wp.tile([P, P], dt.float32, tag="wt")
                    nc.sync.dma_start(out=wt[:], in_=W1[kcb * P : (kcb + 1) * P, :])
                    xt = xp.tile([P, 512], dt.float32, tag="xt")
                    nc.sync.dma_start(out=xt[:], in_=xT[kcb * P : (kcb + 1) * P,
                                                        nb * 512 : (nb + 1) * 512])
                    nc.tensor.matmul(out=hps[:], lhsT=wt[:], rhs=xt[:],
                                     start=(kcb == 0), stop=(kcb == 64))
                h_sb = sb.tile([P, 512], dt.float32, tag="h_sb")
                nc.vector.tensor_copy(out=h_sb[:], in_=hps[:])
                sada_ps = ps.tile([16, 512], dt.float32, space="PSUM", tag="pp_a")
                nc.tensor.matmul(out=sada_ps[0:16, :], lhsT=mb_t[:], rhs=h_sb[:],
                                 start=True, stop=True)
                sada_sb = sb.tile([16, 512], dt.float32, tag="sada_sb")
                nc.vector.tensor_copy(out=sada_sb[:], in_=sada_ps[0:16, :])
                for b in range(4):
                    blk = nb * 4 + b
                    ht_ps = ps.tile([P, P], dt.float32, space="PSUM", tag="pp_b")
                    nc.tensor.matmul(out=ht_ps[:], lhsT=h_sb[:, b * P : (b + 1) * P],
                                     rhs=ident[:], is_transpose=True, start=True, stop=True)
                    st_ps = ps.tile([P, 16], dt.float32, space="PSUM", tag="pp_c")
                    nc.tensor.matmul(out=st_ps[:], lhsT=sada_sb[:, b * P : (b + 1) * P],
                                     rhs=ident[0:16, 0:16], is_transpose=True,
                                     start=True, stop=True)
                    pk = sb.tile([P, 192], dt.float32, tag="pk")
                    nc.vector.tensor_copy(out=pk[:, 0:128], in_=ht_ps[:])
                    nc.vector.tensor_copy(out=pk[:, 128:144], in_=st_ps[:])
                    nc.vector.memset(pk[:, 144:192], 0.0)
                    nc.sync.dma_start(out=Tout[blk * P : (blk + 1) * P, :], in_=pk[:])
    nc.finalize()
    return nc


def build_p23(K, with_next, with_head):
    """P2 (with_next): edge agg + LN/ReLU/residual + W@ + sada + pack.
    P3 (with_head): edge agg + LN/ReLU/residual + row-norm + MLP head."""
    SK = int(sum(K))
    offs = np.cumsum([0] + list(K))
    nc = bacc.Bacc(None, target_bir_lowering=False)
    Tfull = nc.declare_dram_parameter("Tfull", [N, 192], dt.float32, isOutput=False)
    Town = nc.declare_dram_parameter("Town", [1024, 192], dt.float32, isOutput=False)
    xprev = nc.declare_dram_parameter("xprev", [P, 1024], dt.float32, isOutput=False)
    idxq = nc.declare_dram_parameter("idxq", [16, SK * 8], dt.int16, isOutput=False)
    maskq = nc.declare_dram_parameter("maskq", [P, SK], dt.float32, isOutput=False)
    bprev = nc.declare_dram_parameter("bprev", [P, 1], dt.float32, isOutput=False)
    gam = nc.declare_dram_parameter("gam", [P, 1], dt.float32, isOutput=False)
    bet = nc.declare_dram_parameter("bet", [P, 1], dt.float32, isOutput=False)
    rep16q = nc.declare_dram_parameter("rep16q", [8, P], dt.float32, isOutput=False)
    if with_next:
        Wn = nc.declare_dram_parameter("Wn", [P, P], dt.float32, isOutput=False)
        Mb = nc.declare_dram_parameter("Mb", [P, 16], dt.float32, isOutput=False)
        Tout = nc.declare_dram_parameter("Tout", [1024, 192], dt.float32, isOutput=True)
        xnout = nc.declare_dram_parameter("xnout", [P, 1024], dt.float32, isOutput=True)
    if with_head:
        aW1 = nc.declare_dram_parameter("aW1", [P, P], dt.float32, isOutput=False)
        ab1 = nc.declare_dram_parameter("ab1", [P, 1], dt.float32, isOutput=False)
        agm = nc.declare_dram_parameter("agm", [P, 1], dt.float32, isOutput=False)
        abe = nc.declare_dram_parameter("abe", [P, 1], dt.float32, isOutput=False)
        aW2 = nc.declare_dram_parameter("aW2", [P, 1], dt.float32, isOutput=False)
        ab2 = nc.declare_dram_parameter("ab2", [1, 1], dt.float32, isOutput=False)
        rW1 = nc.declare_dram_parameter("rW1", [P, 64], dt.float32, isOutput=False)
        rb1 = nc.declare_dram_parameter("rb1", [64, 1], dt.float32, isOutput=False)
        rgm = nc.declare_dram_parameter("rgm", [64, 1], dt.float32, isOutput=False)
        rbe = nc.declare_dram_parameter("rbe", [64, 1], dt.float32, isOutput=False)
        rW2 = nc.declare_dram_parameter("rW2", [64, 1], dt.float32, isOutput=False)
        rb2 = nc.declare_dram_parameter("rb2", [1, 1], dt.float32, isOutput=False)
        ang = nc.declare_dram_parameter("ang", [1, 1024], dt.float32, isOutput=True)
        rad = nc.declare_dram_parameter("rad", [1, 1024], dt.float32, isOutput=True)

    with tile.TileContext(nc) as tc:
        with (
            tc.tile_pool(name="consts", bufs=1) as consts,
            tc.tile_pool(name="gpool", bufs=3) as gpool,
            tc.tile_pool(name="wpool", bufs=2) as wpool,
            tc.tile_pool(name="sb", bufs=1) as sb,
            tc.tile_pool(name="ps", bufs=2, space="PSUM") as ps,
            tc.tile_pool(name="psagg", bufs=2, space="PSUM") as psagg,
        ):
            c = _mk_consts(nc, consts)
            rep16_t = consts.tile([8, P], dt.float32)
            nc.sync.dma_start(out=rep16_t[:], in_=rep16q[:])

            idx_t = sb.tile([P, SK * 8], dt.int16, tag="idx")
            for a in range(8):
                nc.sync.dma_start(out=idx_t[16 * a : 16 * (a + 1), :], in_=idxq[:])
            mask_t = sb.tile([P, SK], dt.float32, tag="mask")
            nc.sync.dma_start(out=mask_t[:], in_=maskq[:])
            da_t = sb.tile([P, NSTRIPE, 8], dt.float32, tag="da")
            nc.sync.dma_start(
                out=da_t[:],
                in_=Town[:].rearrange("(t p) r -> p t r", p=P)[:, :, 136:144])
            xprev_t = sb.tile([P, 1024], dt.float32, tag="xprev")
            nc.sync.dma_start(out=xprev_t[:], in_=xprev[:])
            bias_t = sb.tile([P, 1], dt.float32, tag="bias")
            nc.sync.dma_start(out=bias_t[:], in_=bprev[:])
            gam_t = sb.tile([P, 1], dt.float32, tag="gam")
            nc.sync.dma_start(out=gam_t[:], in_=gam[:])
            bet_t = sb.tile([P, 1], dt.float32, tag="bet")
            nc.sync.dma_start(out=bet_t[:], in_=bet[:])
            if with_next:
                wn_t = sb.tile([P, P], dt.float32, tag="wn")
                nc.sync.dma_start(out=wn_t[:], in_=Wn[:])
                mb_t = sb.tile([P, 16], dt.float32, tag="mb")
                nc.sync.dma_start(out=mb_t[:], in_=Mb[:])

            xnext = sb.tile([P, 1024], dt.float32, tag="xnext")

            for t in range(NSTRIPE):
                agg_sb = sb.tile([P, P], dt.float32, tag="agg_sb")
                _edge_stripe(nc, c, sb, gpool, wpool, ps, psagg, Tfull, idx_t, mask_t,
                             da_t[:, t, :], K[t], int(offs[t]), agg_sb[:], rep16_t)
                xb = sb.tile([P, P], dt.float32, tag="xb")
                nc.scalar.activation(out=xb[:], in_=agg_sb[:], func=AF.Identity,
                                     bias=bias_t[:], scale=1.0)
                xo = sb.tile([P, P], dt.float32, tag="xo")
                _ln_relu_fm(nc, sb, ps, c, xb[:], P, gam_t, bet_t, xo[:])
                nc.vector.tensor_tensor(out=xnext[:, t * P : (t + 1) * P], in0=xo[:],
                                        in1=xprev_t[:, t * P : (t + 1) * P], op=OP.add)

                if with_next:
                    hn_ps = ps.tile([P, P], dt.float32, space="PSUM", tag="pp_a")
                    nc.tensor.matmul(out=hn_ps[:], lhsT=wn_t[:],
                                     rhs=xnext[:, t * P : (t + 1) * P],
                                     start=True, stop=True)
                    hn_sb = sb.tile([P, P], dt.float32, tag="hn_sb")
                    nc.vector.tensor_copy(out=hn_sb[:], in_=hn_ps[:])
                    sada_ps = ps.tile([16, P], dt.float32, space="PSUM", tag="pp_b")
                    nc.tensor.matmul(out=sada_ps[0:16, :], lhsT=mb_t[:], rhs=hn_sb[:],
                                     start=True, stop=True)
                    sada_sb = sb.tile([16, P], dt.float32, tag="sada_sb")
                    nc.vector.tensor_copy(out=sada_sb[:], in_=sada_ps[0:16, :])
                    ht_ps = ps.tile([P, P], dt.float32, space="PSUM", tag="pp_a")
                    nc.tensor.matmul(out=ht_ps[:], lhsT=hn_sb[:], rhs=c["ident"][:],
                                     is_transpose=True, start=True, stop=True)
                    st_ps = ps.tile([P, 16], dt.float32, space="PSUM", tag="pp_b")
                    nc.tensor.matmul(out=st_ps[:], lhsT=sada_sb[:],
                                     rhs=c["ident"][0:16, 0:16], is_transpose=True,
                                     start=True, stop=True)
                    pk = sb.tile([P, 192], dt.float32, tag="pk")
                    nc.vector.tensor_copy(out=pk[:, 0:128], in_=ht_ps[:])
                    nc.vector.tensor_copy(out=pk[:, 128:144], in_=st_ps[:])
                    nc.vector.memset(pk[:, 144:192], 0.0)
                    nc.sync.dma_start(out=Tout[t * P : (t + 1) * P, :], in_=pk[:])

            if with_next:
                nc.sync.dma_start(out=xnout[:], in_=xnext[:])

            if with_head:
                n = 1024
                xsq = sb.tile([P, n], dt.float32, tag="hd_xsq")
                nc.scalar.activation(out=xsq[:], in_=xnext[:], func=AF.Square)
                h3n = sb.tile([P, n], dt.float32, tag="hd_h3n")
                for j in range(0, n, 512):
                    ss_ps = ps.tile([1, 512], dt.float32, space="PSUM", tag="pp_a")
                    nc.tensor.matmul(out=ss_ps[0:1, :], lhsT=c["ones_col"][:],
                                     rhs=xsq[:, j : j + 512], start=True, stop=True)
                    ss = sb.tile([1, 512], dt.float32, tag="hd_ss")
                    nc.vector.tensor_scalar_max(ss[:], ss_ps[0:1, :], 1e-24)
                    rn = sb.tile([1, 512], dt.float32, tag="hd_rn")
                    _rstd(nc, sb, ss[:], rn[:], 512, 0)
                    rn_rep = ps.tile([P, 512], dt.float32, space="PSUM", tag="pp_b")
                    nc.tensor.matmul(out=rn_rep[:], lhsT=c["ones_row"][:], rhs=rn[:],
                                     start=True, stop=True)
                    nc.vector.tensor_tensor(out=h3n[:, j : j + 512], in0=xnext[:, j : j + 512],
                                            in1=rn_rep[:], op=OP.mult)

                def mm_bias_act(lhsT_t, rhs_sb, m, bias_ap, out_sb):
                    for j in range(0, n, 512):
                        mm_ps = ps.tile([P, 512], dt.float32, space="PSUM", tag="pp_a")
                        nc.tensor.matmul(out=mm_ps[0:m, :], lhsT=lhsT_t,
                                         rhs=rhs_sb[:, j : j + 512], start=True, stop=True)
                        nc.scalar.activation(out=out_sb[:, j : j + 512], in_=mm_ps[0:m, :],
                                             func=AF.Identity, bias=bias_ap, scale=1.0)

                aW1_t = sb.tile([P, P], dt.float32, tag="hd_aW1")
                nc.sync.dma_start(out=aW1_t[:], in_=aW1[:])
                ab1_t = sb.tile([P, 1], dt.float32, tag="hd_ab1")
                nc.sync.dma_start(out=ab1_t[:], in_=ab1[:])
                agm_t = sb.tile([P, 1], dt.float32, tag="hd_agm")
                nc.sync.dma_start(out=agm_t[:], in_=agm[:])
                abe_t = sb.tile([P, 1], dt.float32, tag="hd_abe")
                nc.sync.dma_start(out=abe_t[:], in_=abe[:])
                a_pre = sb.tile([P, n], dt.float32, tag="hd_apre")
                mm_bias_act(aW1_t[:], h3n, P, ab1_t[:], a_pre)
                a_hid = sb.tile([P, n], dt.float32, tag="hd_ahid")
                _ln_relu_fm(nc, sb, ps, c, a_pre[:], n, agm_t, abe_t, a_hid[:])

                aW2_t = sb.tile([P, 1], dt.float32, tag="hd_aW2")
                nc.sync.dma_start(out=aW2_t[:], in_=aW2[:])
                ab2_t = sb.tile([1, 1], dt.float32, tag="hd_ab2")
                nc.sync.dma_start(out=ab2_t[:], in_=ab2[:])
                av = sb.tile([1, n], dt.float32, tag="hd_av")
                mm_bias_act(aW2_t[:], a_hid, 1, ab2_t[:], av)
                # angles = pi*tanh(av) = pi - 2pi/(exp(2av)+1)
                e2 = sb.tile([1, n], dt.float32, tag="hd_e2")
                nc.scalar.activation(out=e2[:], in_=av[:], func=AF.Exp, scale=2.0)
                nc.vector.tensor_scalar_add(e2[:], e2[:], 1.0)
                rr = sb.tile([1, n], dt.float32, tag="hd_rr")
                nc.vector.reciprocal(out=rr[:], in_=e2[:])
                angv = sb.tile([1, n], dt.float32, tag="hd_angv")
                nc.vector.tensor_scalar(out=angv[:], in0=rr[:], scalar1=-2.0 * PI,
                                        scalar2=PI, op0=OP.mult, op1=OP.add)
                nc.sync.dma_start(out=ang[:], in_=angv[:])

                rW1_t = sb.tile([P, 64], dt.float32, tag="hd_rW1")
                nc.sync.dma_start(out=rW1_t[:], in_=rW1[:])
                rb1_t = sb.tile([64, 1], dt.float32, tag="hd_rb1")
                nc.sync.dma_start(out=rb1_t[:], in_=rb1[:])
                rgm_t = sb.tile([64, 1], dt.float32, tag="hd_rgm")
                nc.sync.dma_start(out=rgm_t[:], in_=rgm[:])
                rbe_t = sb.tile([64, 1], dt.float32, tag="hd_rbe")
                nc.sync.dma_start(out=rbe_t[:], in_=rbe[:])
                r_pre = sb.tile([64, n], dt.float32, tag="hd_rpre")
                mm_bias_act(rW1_t[:], h3n, 64, rb1_t[:], r_pre)
                r_hid = sb.tile([64, n], dt.float32, tag="hd_rhid")
                _ln_relu_fm(nc, sb, ps, c, r_pre[:], n, rgm_t, rbe_t, r_hid[:], nfeat=64)

                rW2_t = sb.tile([64, 1], dt.float32, tag="hd_rW2")
                nc.sync.dma_start(out=rW2_t[:], in_=rW2[:])
                rb2_t = sb.tile([1, 1], dt.float32, tag="hd_rb2")
                nc.sync.dma_start(out=rb2_t[:], in_=rb2[:])
                rv = sb.tile([1, n], dt.float32, tag="hd_rv")
                for j in range(0, n, 512):
                    mm_ps = ps.tile([1, 512], dt.float32, space="PSUM", tag="pp_a")
                    nc.tensor.matmul(out=mm_ps[0:1, :], lhsT=rW2_t[:],
                                     rhs=r_hid[:, j : j + 512], start=True, stop=True)
                    nc.scalar.activation(out=rv[:, j : j + 512], in_=mm_ps[0:1, :],
                                         func=AF.Identity, bias=rb2_t[:], scale=1.0)
                # softplus then radius = 1 + 0.1 tanh(sp) = 1.1 - 0.2/(exp(2 sp)+1)
                sp = sb.tile([1, n], dt.float32, tag="hd_sp")
                nc.scalar.activation(out=sp[:], in_=rv[:], func=AF.Exp)
                nc.vector.tensor_scalar_add(sp[:], sp[:], 1.0)
                nc.scalar.activation(out=sp[:], in_=sp[:], func=AF.Ln)
                e2r = sb.tile([1, n], dt.float32, tag="hd_e2r")
                nc.scalar.activation(out=e2r[:], in_=sp[:], func=AF.Exp, scale=2.0)
                nc.vector.tensor_scalar_add(e2r[:], e2r[:], 1.0)
                rr2 = sb.tile([1, n], dt.float32, tag="hd_rr2")
                nc.vector.reciprocal(out=rr2[:], in_=e2r[:])
                radv = sb.tile([1, n], dt.float32, tag="hd_radv")
                nc.vector.tensor_scalar(out=radv[:], in0=rr2[:], scalar1=-0.2,
                                        scalar2=1.1, op0=OP.mult, op1=OP.add)
                nc.sync.dma_start(out=rad[:], in_=radv[:])
    nc.finalize()
    return nc


def build_p4():
    nc = bacc.Bacc(None, target_bir_lowering=False)
    ANG = nc.declare_dram_parameter("ANG", [P, 64], dt.float32, isOutput=False)
    RAD = nc.declare_dram_parameter("RAD", [P, 64], dt.float32, isOutput=False)
    CX = nc.declare_dram_parameter("CX", [P, 64], dt.float32, isOutput=True)
    CY = nc.declare_dram_parameter("CY", [P, 64], dt.float32, isOutput=True)
    with tile.TileContext(nc) as tc:
        with (
            tc.tile_pool(name="consts", bufs=1) as consts,
            tc.tile_pool(name="sb", bufs=1) as sb,
            tc.tile_pool(name="ps", bufs=1, space="PSUM") as ps,
        ):
            ones_col = consts.tile([P, 1], dt.float32)
            nc.gpsimd.memset(ones_col[:], 1.0)
            ones_row = consts.tile([1, P], dt.float32)
            nc.gpsimd.memset(ones_row[:], 1.0)
            half_pi = consts.tile([P, 1], dt.float32)
            nc.gpsimd.memset(half_pi[:], PI / 2.0)

            ang_t = sb.tile([P, 64], dt.float32)
            nc.sync.dma_start(out=ang_t[:], in_=ANG[:])
            rad_t = sb.tile([P, 64], dt.float32)
            nc.sync.dma_start(out=rad_t[:], in_=RAD[:])
            absang = sb.tile([P, 64], dt.float32)
            nc.scalar.activation(out=absang[:], in_=ang_t[:], func=AF.Abs)
            cosx = sb.tile([P, 64], dt.float32)
            nc.scalar.activation(out=cosx[:], in_=absang[:], func=AF.Sin,
                                 scale=-1.0, bias=half_pi[:])
            sinx = sb.tile([P, 64], dt.float32)
            nc.scalar.activation(out=sinx[:], in_=ang_t[:], func=AF.Sin)
            cx = sb.tile([P, 64], dt.float32)
            nc.vector.tensor_tensor(out=cx[:], in0=rad_t[:], in1=cosx[:], op=OP.mult)
            cy = sb.tile([P, 64], dt.float32)
            nc.vector.tensor_tensor(out=cy[:], in0=rad_t[:], in1=sinx[:], op=OP.mult)
            colsum = sb.tile([P, 2], dt.float32)
            nc.vector.tensor_reduce(out=colsum[:, 0:1], in_=cx[:],
                                    axis=mybir.AxisListType.X, op=OP.add)
            nc.vector.tensor_reduce(out=colsum[:, 1:2], in_=cy[:],
                                    axis=mybir.AxisListType.X, op=OP.add)
            tot_ps = ps.tile([1, 2], dt.float32, space="PSUM")
            nc.tensor.matmul(out=tot_ps[0:1, :], lhsT=ones_col[:], rhs=colsum[:],
                             start=True, stop=True)
            mean = sb.tile([1, 2], dt.float32)
            nc.vector.tensor_scalar_mul(mean[:], tot_ps[0:1, :], 1.0 / N)
            mean_rep = ps.tile([P, 2], dt.float32, space="PSUM")
            nc.tensor.matmul(out=mean_rep[:], lhsT=ones_row[:], rhs=mean[:],
                             start=True, stop=True)
            mrep_sb = sb.tile([P, 2], dt.float32)
            nc.vector.tensor_copy(out=mrep_sb[:], in_=mean_rep[:])
            nc.vector.tensor_tensor(out=cx[:], in0=cx[:],
                                    in1=mrep_sb[:, 0:1].to_broadcast([P, 64]),
                                    op=OP.subtract)
            nc.vector.tensor_tensor(out=cy[:], in0=cy[:],
                                    in1=mrep_sb[:, 1:2].to_broadcast([P, 64]),
                                    op=OP.subtract)
            q = sb.tile([P, 64], dt.float32)
            nc.vector.tensor_tensor(out=q[:], in0=cx[:], in1=cx[:], op=OP.mult)
            cy2 = sb.tile([P, 64], dt.float32)
            nc.vector.tensor_tensor(out=cy2[:], in0=cy[:], in1=cy[:], op=OP.mult)
            nc.vector.tensor_tensor(out=q[:], in0=q[:], in1=cy2[:], op=OP.add)
            nc.vector.tensor_scalar_max(q[:], q[:], 1e-24)
            # rsqrt: exp(-0.5 ln q) seed + one Newton polish (table accuracy)
            lnq = sb.tile([P, 64], dt.float32)
            nc.scalar.activation(out=lnq[:], in_=q[:], func=AF.Ln)
            y = sb.tile([P, 64], dt.float32)
            nc.scalar.activation(out=y[:], in_=lnq[:], func=AF.Exp, scale=-0.5)
            u = sb.tile([P, 64], dt.float32)
            for _ in range(2):
                nc.vector.tensor_tensor(out=u[:], in0=y[:], in1=y[:], op=OP.mult)
                nc.vector.tensor_tensor(out=u[:], in0=u[:], in1=q[:], op=OP.mult)
                nc.vector.tensor_scalar(out=u[:], in0=u[:], scalar1=-0.5, scalar2=1.5,
                                        op0=OP.mult, op1=OP.add)
                nc.vector.tensor_tensor(out=y[:], in0=y[:], in1=u[:], op=OP.mult)
            nc.vector.tensor_tensor(out=cx[:], in0=cx[:], in1=y[:], op=OP.mult)
            nc.vector.tensor_tensor(out=cy[:], in0=cy[:], in1=y[:], op=OP.mult)
            nc.sync.dma_start(out=CX[:], in_=cx[:])
            nc.sync.dma_start(out=CY[:], in_=cy[:])
    nc.finalize()
    return nc


# ----------------------------------------------------------------------------
# orchestration
# ----------------------------------------------------------------------------

_REP16 = np.zeros((8, P), np.float32)
for _h in range(8):
    _REP16[_h, _h * 16 : (_h + 1) * 16] = 1.0


def _launch(prog, in_maps, cores, label=""):
    """Run one program on HW; optionally also CoreSim core-0 for a timing
    estimate (GAT_SIMT=1). Appends time (sim if available, else HW) to
    kernel._last_times."""
    from concourse.bass_utils import run_bass_kernel_spmd

    r = run_bass_kernel_spmd(prog, in_maps, cores)
    t = r.exec_time_ns
    if os.environ.get("GAT_SIMT"):
        import time as _time

        from concourse.bass_interp import CoreSim

        sim = CoreSim(prog, require_finite=False, require_nnan=False)
        for k, v in in_maps[0].items():
            sim.tensor(k)[:] = v
        w0 = _time.time()
        sim.simulate()
        t = sim.time
        print(f"  [simt] {label}: {t} ns (sim wall {_time.time()-w0:.1f}s)")
    kernel._last_times.append(t)
    return r


def kernel(**inputs):
    kernel._last_times = []
    x = np.ascontiguousarray(np.asarray(inputs["x"], np.float32))
    prep = host_prep(inputs["src"], inputs["dst"])
    order, K = prep["order"], prep["K"]
    cores = list(range(NCORES))

    xT = np.zeros((INP, N), np.float32)
    xT[:IN] = x[order].T
    W1p = np.zeros((INP, HC), np.float32)
    W1p[:IN] = np.asarray(inputs["W1"], np.float32)
    Mb = {l: mboth(np.asarray(inputs[f"as{l}"], np.float32),
                   np.asarray(inputs[f"ad{l}"], np.float32)) for l in (1, 2, 3)}
    cols = [core_cols(c) for c in cores]

    # ---- P1 ----
    p1 = build_p1()
    in_maps = [dict(xT=np.ascontiguousarray(xT[:, cols[c]]), W1=W1p, Mb=Mb[1])
               for c in cores]
    r1 = _launch(p1, in_maps, cores, "P1")
    Tfull = np.zeros((N, 192), np.float32)
    for c in cores:
        Tfull[cols[c]] = r1.results[c]["Tout"]

    # ---- P2 (layers 2, 3) ----
    p2 = build_p23(K, with_next=True, with_head=False)
    xprev = [np.zeros((P, 1024), np.float32) for _ in cores]
    for l in (2, 3):
        in_maps = []
        for c in cores:
            in_maps.append(dict(
                Tfull=Tfull, Town=np.ascontiguousarray(Tfull[cols[c]]),
                xprev=xprev[c], idxq=prep["idxq"][c], maskq=prep["maskq"][c],
                bprev=np.asarray(inputs[f"b{l-1}"], np.float32).reshape(P, 1),
                gam=np.asarray(inputs[f"g{l-1}"], np.float32).reshape(P, 1),
                bet=np.asarray(inputs[f"be{l-1}"], np.float32).reshape(P, 1),
                Wn=np.ascontiguousarray(np.asarray(inputs[f"W{l}"], np.float32)),
                Mb=Mb[l], rep16q=_REP16,
            ))
        r2 = _launch(p2, in_maps, cores, f"P2.l{l}")
        Tn = np.zeros((N, 192), np.float32)
        for c in cores:
            Tn[cols[c]] = r2.results[c]["Tout"]
            xprev[c] = r2.results[c]["xnout"]
        Tfull = Tn

    # ---- P3 (layer-3 aggregation + MLP head) ----
    p3 = build_p23(K, with_next=False, with_head=True)
    in_maps = []
    for c in cores:
        in_maps.append(dict(
            Tfull=Tfull, Town=np.ascontiguousarray(Tfull[cols[c]]),
            xprev=xprev[c], idxq=prep["idxq"][c], maskq=prep["maskq"][c],
            bprev=np.asarray(inputs["b3"], np.float32).reshape(P, 1),
            gam=np.asarray(inputs["g3"], np.float32).reshape(P, 1),
            bet=np.asarray(inputs["be3"], np.float32).reshape(P, 1),
            rep16q=_REP16,
            aW1=np.ascontiguousarray(np.asarray(inputs["aW1"], np.float32)),
            ab1=np.asarray(inputs["ab1"], np.float32).reshape(P, 1),
            agm=np.asarray(inputs["ag"], np.float32).reshape(P, 1),
            abe=np.asarray(inputs["abe"], np.float32).reshape(P, 1),
            aW2=np.asarray(inputs["aW2"], np.float32).reshape(P, 1),
            ab2=np.asarray(inputs["ab2"], np.float32).reshape(1, 1),
            rW1=np.ascontiguousarray(np.asarray(inputs["rW1"], np.float32)),
            rb1=np.asarray(inputs["rb1"], np.float32).reshape(64, 1),
            rgm=np.asarray(inputs["rg"], np.float32).reshape(64, 1),
            rbe=np.asarray(inputs["rbe"], np.float32).reshape(64, 1),
            rW2=np.asarray(inputs["rW2"], np.float32).reshape(64, 1),
            rb2=np.asarray(inputs["rb2"], np.float32).reshape(1, 1),
        ))
    r3 = _launch(p3, in_maps, cores, "P3")
    ang = np.zeros(N, np.float32)
    rad = np.zeros(N, np.float32)
    for c in cores:
        ang[cols[c]] = r3.results[c]["ang"][0]
        rad[cols[c]] = r3.results[c]["rad"][0]

    # ---- P4 (finalize, replicated) ----
    p4 = build_p4()
    r4 = _launch(
        p4, [dict(ANG=ang.reshape(P, 64), RAD=rad.reshape(P, 64))] * NCORES, cores,
        "P4")
    cxv = r4.results[0]["CX"].reshape(N)
    cyv = r4.results[0]["CY"].reshape(N)

    out = np.zeros((N, 2), np.float32)
    out[order, 0] = cxv
    out[order, 1] = cyv
    return out



# revision 20
# speedup vs baseline: 1.1213x; 1.1213x over previous
"""Trainium2 Bass kernel for nn_GAT_86045374808682 (3-layer GAT + coordinate head).

Self-contained: takes FULL inputs, shards across 8 NeuronCores internally,
returns the FULL [8192, 2] float32 output.

v1 optimizations over baseline:
- fp16 512-B table rows [h(128)|sa(8)|da(8)|pad] -> gather descriptors 22.76ns
  (was 768B/34.1ns); ghost-node padding replaces additive masks entirely.
- fp16 DVE edge math (2x mode), fp16 PE transposes (1 cyc/row), fp32r bitcast
  for fp32 matmuls with free dim >= 256 (1 cyc/row vs 4).
- No Ln activations anywhere: rsqrt = DVE bit-trick + 2 Newton steps;
  tanh(softplus(x)) = 1 - 2/((1+e^x)^2+1). All ACT funcs in one table set
  (Exp/Tanh/Relu/Square/Identity/Abs) -> zero act-table reloads.
- W1 host-swizzled [128, 65*128] fp16, loaded in one DMA; x shipped fp16.
"""
import os
import sys

import numpy as np

for _p in ("/opt/trn_rl_repo", "/root/.axon_site/_ro/trn_rl_repo"):
    if _p not in sys.path:
        sys.path.append(_p)

import concourse.bass as bass  # noqa: F401
import concourse.tile as tile
from concourse import bacc, library_config, mybir
from concourse.masks import make_identity

dt = mybir.dt
AF = mybir.ActivationFunctionType
OP = mybir.AluOpType

N = 8192
IN = 8193
INP = 8320  # 65 * 128
H = 8
HC = 128
P = 128
NCORES = 8
NSTRIPE = 8
KC = 32  # gather chunk (slots)
ROW = 256  # fp16 elems per table row (512 B)
NT = N + 2  # table rows (ghost at N)
GHOST_SA = -30000.0
PI = float(np.pi)
MAGIC = 0x5F3759DF


# ----------------------------------------------------------------------------
# host-side graph prep
# ----------------------------------------------------------------------------

def host_prep(src, dst):
    s = np.concatenate([np.asarray(src).astype(np.int64), np.arange(N, dtype=np.int64)])
    d = np.concatenate([np.asarray(dst).astype(np.int64), np.arange(N, dtype=np.int64)])
    deg = np.bincount(d, minlength=N)
    order = np.argsort(-deg, kind="stable")  # new-id -> old-id
    old2new = np.empty(N, np.int64)
    old2new[order] = np.arange(N)
    s_new = old2new[s]
    d_new = old2new[d]
    deg_new = deg[order]

    K = [int(deg_new[1024 * t]) for t in range(NSTRIPE)]  # desc-sorted -> stripe max
    offs = np.cumsum([0] + K)

    eo = np.argsort(d_new, kind="stable")
    s_sorted = s_new[eo]
    starts = np.searchsorted(d_new[eo], np.arange(N))

    idxq = np.zeros((NCORES, 16, int(offs[-1]) * 8), np.int16)
    ar = np.arange(P)
    for c in range(NCORES):
        for t in range(NSTRIPE):
            Kt = K[t]
            vids = (t * NCORES + c) * P + ar
            e0 = starts[vids]
            degs = deg_new[vids]
            kk = np.arange(Kt)
            take = np.minimum(e0[:, None] + kk[None, :], len(s_sorted) - 1)
            mat = s_sorted[take]                      # [128, Kt]
            valid = kk[None, :] < degs[:, None]
            mat = np.where(valid, mat, N)             # ghost node for padding
            lin = mat.T.reshape(-1)                   # slot-major [Kt*128]
            o16 = int(offs[t]) * 8
            idxq[c, :, o16 : o16 + Kt * 8] = lin.reshape(-1, 16).T
    return dict(order=order, K=K, offs=offs, idxq=idxq.astype(np.int16))


def core_cols(c):
    return np.concatenate([np.arange((t * NCORES + c) * P, (t * NCORES + c) * P + P)
                           for t in range(NSTRIPE)])


def mboth(a_src, a_dst):
    M = np.zeros((P, 16), np.float32)
    for h in range(H):
        M[h * 16 : (h + 1) * 16, h] = a_src[h]
        M[h * 16 : (h + 1) * 16, 8 + h] = a_dst[h]
    return M


# ----------------------------------------------------------------------------
# shared bass building blocks
# ----------------------------------------------------------------------------

def _mk_consts(nc, consts):
    c = {"pool": consts}
    nc.gpsimd.load_library(library_config.mlp)
    c["ident16"] = consts.tile([P, P], dt.float16, name="c_ident16")
    make_identity(nc, c["ident16"][:])
    c["ones_col"] = consts.tile([P, 1], dt.float16, name="c_ones_col")
    nc.gpsimd.memset(c["ones_col"][:], 1.0)
    c["ones_row"] = consts.tile([1, P], dt.float16, name="c_ones_row")
    nc.gpsimd.memset(c["ones_row"][:], 1.0)
    c["magic"] = consts.tile([1, 512], dt.int32, name="c_magic")
    nc.gpsimd.memset(c["magic"][:], MAGIC)
    return c


def _rstd_dve(nc, sb, c, var_ap, out_ap, n, eps):
    """out = 1/sqrt(var+eps), pure DVE: bit-trick seed + 2 Newton steps."""
    vpe = sb.tile([1, 512], dt.float32, tag="rs_vpe")
    if eps:
        nc.vector.tensor_scalar_add(vpe[:, 0:n], var_ap, float(eps))
    else:
        nc.vector.tensor_copy(out=vpe[:, 0:n], in_=var_ap)
    v = vpe[:, 0:n]
    iv = sb.tile([1, 512], dt.int32, tag="rs_iv")
    nc.vector.tensor_scalar(out=iv[:, 0:n], in0=v.bitcast(dt.int32), scalar1=1,
                            scalar2=None, op0=OP.logical_shift_right)
    y = sb.tile([1, 512], dt.float32, tag="rs_y")
    nc.vector.tensor_tensor(out=y[:, 0:n].bitcast(dt.int32), in0=c["magic"][:, 0:n],
                            in1=iv[:, 0:n], op=OP.subtract)
    u = sb.tile([1, 512], dt.float32, tag="rs_u")
    for _ in range(2):
        nc.vector.tensor_tensor(out=u[:, 0:n], in0=y[:, 0:n], in1=y[:, 0:n], op=OP.mult)
        nc.vector.tensor_tensor(out=u[:, 0:n], in0=u[:, 0:n], in1=v, op=OP.mult)
        nc.vector.tensor_scalar(out=u[:, 0:n], in0=u[:, 0:n], scalar1=-0.5, scalar2=1.5,
                                op0=OP.mult, op1=OP.add)
        nc.vector.tensor_tensor(out=y[:, 0:n], in0=y[:, 0:n], in1=u[:, 0:n], op=OP.mult)
    nc.vector.tensor_copy(out=out_ap, in_=y[:, 0:n])


def _ln_relu_fm(nc, sb, ps, c, x_sb, n, gamma_t, beta_t, out_sb, nfeat=P):
    """Feature-major LN + affine + ReLU: out = relu(gamma*(x-mu)*rstd + beta).
    x_sb [nfeat, n] fp32 SBUF; per-column stats; 512-col chunks. Matmul inputs
    go through fp16 copies (1 cyc/row on PE; stats still accumulate fp32)."""
    for j in range(0, n, 512):
        w = min(512, n - j)
        xs = x_sb[:, j : j + w]
        x16 = sb.tile([nfeat, 512], dt.float16, tag="ln_x16")
        nc.vector.tensor_copy(out=x16[:, 0:w], in_=xs)
        xsq = sb.tile([nfeat, 512], dt.float16, tag="ln_xsq")
        nc.scalar.activation(out=xsq[:, 0:w], in_=x16[:, 0:w], func=AF.Square)
        s1_ps = ps.tile([1, 512], dt.float32, space="PSUM", tag="pp_a")
        nc.tensor.matmul(out=s1_ps[:, 0:w], lhsT=c["ones_col"][0:nfeat, :],
                         rhs=x16[:, 0:w], start=True, stop=True)
        s2_ps = ps.tile([1, 512], dt.float32, space="PSUM", tag="pp_b")
        nc.tensor.matmul(out=s2_ps[:, 0:w], lhsT=c["ones_col"][0:nfeat, :],
                         rhs=xsq[:, 0:w], start=True, stop=True)
        mu = sb.tile([1, 512], dt.float32, tag="ln_mu")
        nc.vector.tensor_scalar_mul(mu[:, 0:w], s1_ps[:, 0:w], 1.0 / nfeat)
        musq = sb.tile([1, 512], dt.float32, tag="ln_musq")
        nc.vector.tensor_tensor(out=musq[:, 0:w], in0=mu[:, 0:w], in1=mu[:, 0:w],
                                op=OP.mult)
        var = sb.tile([1, 512], dt.float32, tag="ln_var")
        nc.vector.scalar_tensor_tensor(out=var[:, 0:w], in0=s2_ps[:, 0:w],
                                       scalar=1.0 / nfeat, in1=musq[:, 0:w],
                                       op0=OP.mult, op1=OP.subtract)
        rs = sb.tile([1, 512], dt.float32, tag="ln_rs")
        _rstd_dve(nc, sb, c, var[:, 0:w], rs[:, 0:w], w, 1e-5)
        mu16 = sb.tile([1, 512], dt.float16, tag="ln_mu16")
        nc.vector.tensor_copy(out=mu16[:, 0:w], in_=mu[:, 0:w])
        rs16 = sb.tile([1, 512], dt.float16, tag="ln_rs16")
        nc.vector.tensor_copy(out=rs16[:, 0:w], in_=rs[:, 0:w])
        rep_mu = ps.tile([nfeat, 512], dt.float32, space="PSUM", tag="pp_a")
        nc.tensor.matmul(out=rep_mu[:, 0:w], lhsT=c["ones_row"][:, 0:nfeat],
                         rhs=mu16[:, 0:w], start=True, stop=True)
        rep_rs = ps.tile([nfeat, 512], dt.float32, space="PSUM", tag="pp_b")
        nc.tensor.matmul(out=rep_rs[:, 0:w], lhsT=c["ones_row"][:, 0:nfeat],
                         rhs=rs16[:, 0:w], start=True, stop=True)
        xh = sb.tile([nfeat, 512], dt.float32, tag="ln_xh")
        nc.vector.tensor_tensor(out=xh[:, 0:w], in0=xs, in1=rep_mu[:, 0:w], op=OP.subtract)
        nc.vector.tensor_tensor(out=xh[:, 0:w], in0=xh[:, 0:w], in1=rep_rs[:, 0:w],
                                op=OP.mult)
        nc.scalar.activation(out=out_sb[:, j : j + w], in_=xh[:, 0:w], func=AF.Relu,
                             scale=gamma_t[:], bias=beta_t[:])


def _edge_stripe(nc, c, sb, gpool, wpool, ps, psagg, Tfull, idx_t, da_stripe,
                 K_t, off_t, agg_sb, rep16_t):
    """One stripe: gather + segment softmax + weighted sum for 128 dst lanes.
    Writes normalized aggregation (feature-major [128 f, 128 dst] fp32) to
    agg_sb. Padding edges point at the ghost row (h=0, sa=-3e4) -> ex=0, w=0."""
    nchunk = (K_t + KC - 1) // KC
    agg = psagg.tile([P, P], dt.float32, space="PSUM", tag="agg")
    den = sb.tile([P, 8], dt.float32, tag="den")
    for ci in range(nchunk):
        k0 = ci * KC
        kc = min(KC, K_t - k0)
        g = gpool.tile([P, KC, ROW], dt.float16, tag="gather")
        nc.gpsimd.dma_gather(
            out_ap=g[:, 0:kc, :],
            in_ap=Tfull[:],
            idxs_ap=idx_t[:, (off_t + k0) * 8 : (off_t + k0 + kc) * 8],
            num_idxs=kc * P,
            num_idxs_reg=kc * P,
            elem_size=ROW,
            single_packet=False,
        )
        z = sb.tile([P, KC, 8], dt.float16, tag="z")
        nc.vector.tensor_tensor(out=z[:, 0:kc, :], in0=g[:, 0:kc, 128:136],
                                in1=da_stripe.unsqueeze(1).to_broadcast([P, kc, 8]),
                                op=OP.add)
        zl = sb.tile([P, KC, 8], dt.float16, tag="zl")
        nc.vector.scalar_tensor_tensor(out=zl[:, 0:kc, :], in0=z[:, 0:kc, :],
                                       scalar=0.2, in1=z[:, 0:kc, :],
                                       op0=OP.mult, op1=OP.max)
        ex = sb.tile([P, KC, 8], dt.float16, tag="ex")
        nc.scalar.activation(out=ex[:, 0:kc, :], in_=zl[:, 0:kc, :], func=AF.Exp)
        dc = sb.tile([P, 8], dt.float32, tag="dc")
        nc.vector.tensor_reduce(out=dc[:], in_=ex[:, 0:kc, :].transpose([0, 2, 1]),
                                axis=mybir.AxisListType.X, op=OP.add)
        if ci == 0:
            nc.vector.tensor_copy(out=den[:], in_=dc[:])
        else:
            nc.vector.tensor_tensor(out=den[:], in0=den[:], in1=dc[:], op=OP.add)
        w = wpool.tile([P, KC, P], dt.float16, tag="w")
        nc.vector.tensor_tensor(
            out=w[:, 0:kc, :].rearrange("p k (h e) -> p k h e", h=8),
            in0=g[:, 0:kc, 0:128].rearrange("p k (h e) -> p k h e", h=8),
            in1=ex[:, 0:kc, :].unsqueeze(3).to_broadcast([P, kc, 8, 16]),
            op=OP.mult)
        for k in range(kc):
            # transpose-and-accumulate via regular matmul (w stationary,
            # identity streaming): 16-bit is_transpose PSUM accumulation is
            # broken on HW, regular-matmul accumulation is exact fp32.
            nc.tensor.matmul(out=agg[:], lhsT=w[:, k, :], rhs=c["ident16"][:],
                             start=(ci == 0 and k == 0),
                             stop=(ci == nchunk - 1 and k == kc - 1))
    den16 = sb.tile([P, 8], dt.float16, tag="den16")
    nc.vector.tensor_copy(out=den16[:], in_=den[:])
    dent = ps.tile([8, P], dt.float16, space="PSUM", tag="pp_a")
    nc.tensor.matmul(out=dent[0:8, :], lhsT=den16[:], rhs=c["ident16"][:],
                     is_transpose=True, start=True, stop=True)
    rden = sb.tile([8, P], dt.float16, tag="rden")
    with nc.allow_low_precision(reason="fp16 1/den; den ~O(1-100), plenty for 2e-2 gate"):
        nc.vector.reciprocal(out=rden[:], in_=dent[0:8, :])
    rdrep = ps.tile([P, P], dt.float32, space="PSUM", tag="pp_b")
    nc.tensor.matmul(out=rdrep[:], lhsT=rep16_t[:], rhs=rden[:], start=True, stop=True)
    rdrep_sb = sb.tile([P, P], dt.float32, tag="rdrep_sb")
    nc.vector.tensor_copy(out=rdrep_sb[:], in_=rdrep[:])
    nc.vector.tensor_tensor(out=agg_sb, in0=agg[:], in1=rdrep_sb[:], op=OP.mult)


# ----------------------------------------------------------------------------
# program builders
# ----------------------------------------------------------------------------

def build_p1():
    nc = bacc.Bacc(None, target_bir_lowering=False)
    xT = nc.declare_dram_parameter("xT", [INP, 1024], dt.float16, isOutput=False)
    W1x = nc.declare_dram_parameter("W1x", [P, 65 * P], dt.float16, isOutput=False)
    Mb = nc.declare_dram_parameter("Mb", [P, 16], dt.float16, isOutput=False)
    Tout = nc.declare_dram_parameter("Tout", [1024, ROW], dt.float16, isOutput=True)

    with tile.TileContext(nc) as tc:
        with (
            tc.tile_pool(name="consts", bufs=1) as consts,
            tc.tile_pool(name="xpool", bufs=3) as xp,
            tc.tile_pool(name="sb", bufs=2) as sb,
            tc.tile_pool(name="psh", bufs=2, space="PSUM") as psh,
            tc.tile_pool(name="ps", bufs=2, space="PSUM") as ps,
        ):
            ident16 = consts.tile([P, P], dt.float16)
            make_identity(nc, ident16[:])
            mb_t = consts.tile([P, 16], dt.float16)
            nc.sync.dma_start(out=mb_t[:], in_=Mb[:])
            w1_t = consts.tile([P, 65, P], dt.float16)
            nc.sync.dma_start(out=w1_t[:], in_=W1x[:].rearrange("p (c j) -> p c j", c=65))
            pk = consts.tile([P, ROW], dt.float16)
            nc.vector.memset(pk[:], 0.0)
            for nb in range(2):
                hps = psh.tile([P, 512], dt.float32, space="PSUM", tag="hps")
                for kcb in range(65):
                    xt = xp.tile([P, 512], dt.float16, tag="xt")
                    nc.sync.dma_start(out=xt[:], in_=xT[kcb * P : (kcb + 1) * P,
                                                        nb * 512 : (nb + 1) * 512])
                    nc.tensor.matmul(out=hps[:], lhsT=w1_t[:, kcb, :], rhs=xt[:],
                                     start=(kcb == 0), stop=(kcb == 64))
                h16 = sb.tile([P, 512], dt.float16, tag="h16")
                nc.vector.tensor_copy(out=h16[:], in_=hps[:])
                sada_ps = ps.tile([16, 512], dt.float32, space="PSUM", tag="pp_a")
                nc.tensor.matmul(out=sada_ps[0:16, :], lhsT=mb_t[:], rhs=h16[:],
                                 start=True, stop=True)
                sada16 = sb.tile([16, 512], dt.float16, tag="sada16")
                nc.vector.tensor_copy(out=sada16[:], in_=sada_ps[0:16, :])
                for b in range(4):
                    blk = nb * 4 + b
                    ht_ps = ps.tile([P, P], dt.float16, space="PSUM", tag="pp_b")
                    nc.tensor.matmul(out=ht_ps[:], lhsT=h16[:, b * P : (b + 1) * P],
                                     rhs=ident16[:], is_transpose=True, start=True,
                                     stop=True)
                    st_ps = ps.tile([P, 16], dt.float16, space="PSUM", tag="pp_c")
                    nc.tensor.matmul(out=st_ps[:], lhsT=sada16[:, b * P : (b + 1) * P],
                                     rhs=ident16[0:16, 0:16], is_transpose=True,
                                     start=True, stop=True)
                    nc.vector.tensor_copy(out=pk[:, 0:128], in_=ht_ps[:])
                    nc.vector.tensor_copy(out=pk[:, 128:144], in_=st_ps[:])
                    nc.sync.dma_start(out=Tout[blk * P : (blk + 1) * P, :], in_=pk[:])
    nc.finalize()
    return nc


def build_p23(K, with_next, with_head):
    """P2 (with_next): edge agg + LN/ReLU/residual + W@ + sada + pack.
    P3 (with_head): edge agg + LN/ReLU/residual + row-norm + MLP head."""
    SK = int(sum(K))
    offs = np.cumsum([0] + list(K))
    nc = bacc.Bacc(None, target_bir_lowering=False)
    Tfull = nc.declare_dram_parameter("Tfull", [NT, ROW], dt.float16, isOutput=False)
    Town = nc.declare_dram_parameter("Town", [1024, ROW], dt.float16, isOutput=False)
    xprev = nc.declare_dram_parameter("xprev", [P, 1024], dt.float32, isOutput=False)
    idxq = nc.declare_dram_parameter("idxq", [16, SK * 8], dt.int16, isOutput=False)
    bprev = nc.declare_dram_parameter("bprev", [P, 1], dt.float32, isOutput=False)
    gam = nc.declare_dram_parameter("gam", [P, 1], dt.float32, isOutput=False)
    bet = nc.declare_dram_parameter("bet", [P, 1], dt.float32, isOutput=False)
    rep16q = nc.declare_dram_parameter("rep16q", [8, P], dt.float16, isOutput=False)
    if with_next:
        Wn = nc.declare_dram_parameter("Wn", [P, P], dt.float16, isOutput=False)
        Mb = nc.declare_dram_parameter("Mb", [P, 16], dt.float16, isOutput=False)
        Tout = nc.declare_dram_parameter("Tout", [1024, ROW], dt.float16, isOutput=True)
        xnout = nc.declare_dram_parameter("xnout", [P, 1024], dt.float32, isOutput=True)
    if with_head:
        aW1 = nc.declare_dram_parameter("aW1", [P, P], dt.float16, isOutput=False)
        ab1 = nc.declare_dram_parameter("ab1", [P, 1], dt.float32, isOutput=False)
        agm = nc.declare_dram_parameter("agm", [P, 1], dt.float32, isOutput=False)
        abe = nc.declare_dram_parameter("abe", [P, 1], dt.float32, isOutput=False)
        aW2 = nc.declare_dram_parameter("aW2", [P, 1], dt.float16, isOutput=False)
        ab2 = nc.declare_dram_parameter("ab2", [1, 1], dt.float32, isOutput=False)
        rW1 = nc.declare_dram_parameter("rW1", [P, 64], dt.float16, isOutput=False)
        rb1 = nc.declare_dram_parameter("rb1", [64, 1], dt.float32, isOutput=False)
        rgm = nc.declare_dram_parameter("rgm", [64, 1], dt.float32, isOutput=False)
        rbe = nc.declare_dram_parameter("rbe", [64, 1], dt.float32, isOutput=False)
        rW2 = nc.declare_dram_parameter("rW2", [64, 1], dt.float16, isOutput=False)
        rb2 = nc.declare_dram_parameter("rb2", [1, 1], dt.float32, isOutput=False)
        ang = nc.declare_dram_parameter("ang", [1, 1024], dt.float32, isOutput=True)
        rad = nc.declare_dram_parameter("rad", [1, 1024], dt.float32, isOutput=True)

    with tile.TileContext(nc) as tc:
        with (
            tc.tile_pool(name="consts", bufs=1) as consts,
            tc.tile_pool(name="gpool", bufs=3) as gpool,
            tc.tile_pool(name="wpool", bufs=2) as wpool,
            tc.tile_pool(name="sb", bufs=1) as sb,
            tc.tile_pool(name="ps", bufs=2, space="PSUM") as ps,
            tc.tile_pool(name="psagg", bufs=2, space="PSUM") as psagg,
        ):
            c = _mk_consts(nc, consts)
            rep16_t = consts.tile([8, P], dt.float16)
            nc.sync.dma_start(out=rep16_t[:], in_=rep16q[:])

            idx_t = sb.tile([P, SK * 8], dt.int16, tag="idx")
            for a in range(8):
                nc.sync.dma_start(out=idx_t[16 * a : 16 * (a + 1), :], in_=idxq[:])
            da_t = sb.tile([P, NSTRIPE, 8], dt.float16, tag="da")
            nc.sync.dma_start(
                out=da_t[:],
                in_=Town[:].rearrange("(t p) r -> p t r", p=P)[:, :, 136:144])
            xprev_t = sb.tile([P, 1024], dt.float32, tag="xprev")
            nc.sync.dma_start(out=xprev_t[:], in_=xprev[:])
            bias_t = sb.tile([P, 1], dt.float32, tag="bias")
            nc.sync.dma_start(out=bias_t[:], in_=bprev[:])
            gam_t = sb.tile([P, 1], dt.float32, tag="gam")
            nc.sync.dma_start(out=gam_t[:], in_=gam[:])
            bet_t = sb.tile([P, 1], dt.float32, tag="bet")
            nc.sync.dma_start(out=bet_t[:], in_=bet[:])
            if with_next:
                wn_t = sb.tile([P, P], dt.float16, tag="wn")
                nc.sync.dma_start(out=wn_t[:], in_=Wn[:])
                mb_t = sb.tile([P, 16], dt.float16, tag="mb")
                nc.sync.dma_start(out=mb_t[:], in_=Mb[:])
                pk = sb.tile([P, ROW], dt.float16, tag="pk")
                nc.vector.memset(pk[:], 0.0)

            xnext = sb.tile([P, 1024], dt.float32, tag="xnext")

            for t in range(NSTRIPE):
                agg_sb = sb.tile([P, P], dt.float32, tag="agg_sb")
                _edge_stripe(nc, c, sb, gpool, wpool, ps, psagg, Tfull, idx_t,
                             da_t[:, t, :], K[t], int(offs[t]), agg_sb[:], rep16_t)
                xb = sb.tile([P, P], dt.float32, tag="xb")
                nc.scalar.activation(out=xb[:], in_=agg_sb[:], func=AF.Identity,
                                     bias=bias_t[:], scale=1.0)
                xo = sb.tile([P, P], dt.float32, tag="xo")
                _ln_relu_fm(nc, sb, ps, c, xb[:], P, gam_t, bet_t, xo[:])
                nc.vector.tensor_tensor(out=xnext[:, t * P : (t + 1) * P], in0=xo[:],
                                        in1=xprev_t[:, t * P : (t + 1) * P], op=OP.add)

                if with_next:
                    xs16 = sb.tile([P, P], dt.float16, tag="xs16")
                    nc.vector.tensor_copy(out=xs16[:], in_=xnext[:, t * P : (t + 1) * P])
                    hn_ps = ps.tile([P, P], dt.float32, space="PSUM", tag="pp_a")
                    nc.tensor.matmul(out=hn_ps[:], lhsT=wn_t[:], rhs=xs16[:],
                                     start=True, stop=True)
                    hn16 = sb.tile([P, P], dt.float16, tag="hn16")
                    nc.vector.tensor_copy(out=hn16[:], in_=hn_ps[:])
                    sada_ps = ps.tile([16, P], dt.float32, space="PSUM", tag="pp_b")
                    nc.tensor.matmul(out=sada_ps[0:16, :], lhsT=mb_t[:], rhs=hn16[:],
                                     start=True, stop=True)
                    sada16 = sb.tile([16, P], dt.float16, tag="sada16")
                    nc.vector.tensor_copy(out=sada16[:], in_=sada_ps[0:16, :])
                    ht_ps = ps.tile([P, P], dt.float16, space="PSUM", tag="pp_a")
                    nc.tensor.matmul(out=ht_ps[:], lhsT=hn16[:], rhs=c["ident16"][:],
                                     is_transpose=True, start=True, stop=True)
                    st_ps = ps.tile([P, 16], dt.float16, space="PSUM", tag="pp_b")
                    nc.tensor.matmul(out=st_ps[:], lhsT=sada16[:],
                                     rhs=c["ident16"][0:16, 0:16], is_transpose=True,
                                     start=True, stop=True)
                    nc.vector.tensor_copy(out=pk[:, 0:128], in_=ht_ps[:])
                    nc.vector.tensor_copy(out=pk[:, 128:144], in_=st_ps[:])
                    nc.sync.dma_start(out=Tout[t * P : (t + 1) * P, :], in_=pk[:])

            if with_next:
                nc.sync.dma_start(out=xnout[:], in_=xnext[:])

            if with_head:
                n = 1024
                xn16 = sb.tile([P, n], dt.float16, tag="hd_xn16")
                nc.vector.tensor_copy(out=xn16[:], in_=xnext[:])
                xsq = sb.tile([P, n], dt.float16, tag="hd_xsq")
                nc.scalar.activation(out=xsq[:], in_=xn16[:], func=AF.Square)
                h3n = sb.tile([P, n], dt.float32, tag="hd_h3n")
                for j in range(0, n, 512):
                    ss_ps = ps.tile([1, 512], dt.float32, space="PSUM", tag="pp_a")
                    nc.tensor.matmul(out=ss_ps[0:1, :], lhsT=c["ones_col"][:],
                                     rhs=xsq[:, j : j + 512], start=True, stop=True)
                    ss = sb.tile([1, 512], dt.float32, tag="hd_ss")
                    nc.vector.tensor_scalar_max(ss[:], ss_ps[0:1, :], 1e-24)
                    rn = sb.tile([1, 512], dt.float32, tag="hd_rn")
                    _rstd_dve(nc, sb, c, ss[:], rn[:], 512, 0)
                    rn16 = sb.tile([1, 512], dt.float16, tag="hd_rn16")
                    nc.vector.tensor_copy(out=rn16[:], in_=rn[:])
                    rn_rep = ps.tile([P, 512], dt.float32, space="PSUM", tag="pp_b")
                    nc.tensor.matmul(out=rn_rep[:], lhsT=c["ones_row"][:],
                                     rhs=rn16[:], start=True, stop=True)
                    nc.vector.tensor_tensor(out=h3n[:, j : j + 512], in0=xnext[:, j : j + 512],
                                            in1=rn_rep[:], op=OP.mult)
                h3n16 = sb.tile([P, n], dt.float16, tag="hd_h3n16")
                nc.vector.tensor_copy(out=h3n16[:], in_=h3n[:])

                def mm_bias_act(lhsT_t, rhs16, m, bias_ap, out_sb):
                    for j in range(0, n, 512):
                        mm_ps = ps.tile([P, 512], dt.float32, space="PSUM", tag="pp_a")
                        nc.tensor.matmul(out=mm_ps[0:m, :], lhsT=lhsT_t,
                                         rhs=rhs16[:, j : j + 512], start=True, stop=True)
                        nc.scalar.activation(out=out_sb[:, j : j + 512], in_=mm_ps[0:m, :],
                                             func=AF.Identity, bias=bias_ap, scale=1.0)

                aW1_t = sb.tile([P, P], dt.float16, tag="hd_aW1")
                nc.sync.dma_start(out=aW1_t[:], in_=aW1[:])
                ab1_t = sb.tile([P, 1], dt.float32, tag="hd_ab1")
                nc.sync.dma_start(out=ab1_t[:], in_=ab1[:])
                agm_t = sb.tile([P, 1], dt.float32, tag="hd_agm")
                nc.sync.dma_start(out=agm_t[:], in_=agm[:])
                abe_t = sb.tile([P, 1], dt.float32, tag="hd_abe")
                nc.sync.dma_start(out=abe_t[:], in_=abe[:])
                a_pre = sb.tile([P, n], dt.float32, tag="hd_apre")
                mm_bias_act(aW1_t[:], h3n16, P, ab1_t[:], a_pre)
                a_hid = sb.tile([P, n], dt.float32, tag="hd_ahid")
                _ln_relu_fm(nc, sb, ps, c, a_pre[:], n, agm_t, abe_t, a_hid[:])
                a_hid16 = sb.tile([P, n], dt.float16, tag="hd_ahid16")
                nc.vector.tensor_copy(out=a_hid16[:], in_=a_hid[:])

                aW2_t = sb.tile([P, 1], dt.float16, tag="hd_aW2")
                nc.sync.dma_start(out=aW2_t[:], in_=aW2[:])
                ab2_t = sb.tile([1, 1], dt.float32, tag="hd_ab2")
                nc.sync.dma_start(out=ab2_t[:], in_=ab2[:])
                av = sb.tile([1, n], dt.float32, tag="hd_av")
                mm_bias_act(aW2_t[:], a_hid16, 1, ab2_t[:], av)
                # angles = pi * tanh(av)
                angv = sb.tile([1, n], dt.float32, tag="hd_angv")
                nc.scalar.activation(out=angv[:], in_=av[:], func=AF.Tanh)
                nc.vector.tensor_scalar_mul(angv[:], angv[:], PI)
                nc.sync.dma_start(out=ang[:], in_=angv[:])

                rW1_t = sb.tile([P, 64], dt.float16, tag="hd_rW1")
                nc.sync.dma_start(out=rW1_t[:], in_=rW1[:])
                rb1_t = sb.tile([64, 1], dt.float32, tag="hd_rb1")
                nc.sync.dma_start(out=rb1_t[:], in_=rb1[:])
                rgm_t = sb.tile([64, 1], dt.float32, tag="hd_rgm")
                nc.sync.dma_start(out=rgm_t[:], in_=rgm[:])
                rbe_t = sb.tile([64, 1], dt.float32, tag="hd_rbe")
                nc.sync.dma_start(out=rbe_t[:], in_=rbe[:])
                r_pre = sb.tile([64, n], dt.float32, tag="hd_rpre")
                mm_bias_act(rW1_t[:], h3n16, 64, rb1_t[:], r_pre)
                r_hid = sb.tile([64, n], dt.float32, tag="hd_rhid")
                _ln_relu_fm(nc, sb, ps, c, r_pre[:], n, rgm_t, rbe_t, r_hid[:], nfeat=64)
                r_hid16 = sb.tile([64, n], dt.float16, tag="hd_rhid16")
                nc.vector.tensor_copy(out=r_hid16[:], in_=r_hid[:])

                rW2_t = sb.tile([64, 1], dt.float16, tag="hd_rW2")
                nc.sync.dma_start(out=rW2_t[:], in_=rW2[:])
                rb2_t = sb.tile([1, 1], dt.float32, tag="hd_rb2")
                nc.sync.dma_start(out=rb2_t[:], in_=rb2[:])
                rv = sb.tile([1, n], dt.float32, tag="hd_rv")
                for j in range(0, n, 512):
                    mm_ps = ps.tile([1, 512], dt.float32, space="PSUM", tag="pp_a")
                    nc.tensor.matmul(out=mm_ps[0:1, :], lhsT=rW2_t[:],
                                     rhs=r_hid16[:, j : j + 512], start=True, stop=True)
                    nc.scalar.activation(out=rv[:, j : j + 512], in_=mm_ps[0:1, :],
                                         func=AF.Identity, bias=rb2_t[:], scale=1.0)
                # radius = 1 + 0.1*tanh(softplus(rv)) = 1.1 - 0.2/((1+e^rv)^2 + 1)
                e1 = sb.tile([1, n], dt.float32, tag="hd_e1")
                nc.scalar.activation(out=e1[:], in_=rv[:], func=AF.Exp)
                nc.vector.tensor_scalar_add(e1[:], e1[:], 1.0)
                vsq = sb.tile([1, n], dt.float32, tag="hd_vsq")
                nc.vector.tensor_tensor(out=vsq[:], in0=e1[:], in1=e1[:], op=OP.mult)
                nc.vector.tensor_scalar_add(vsq[:], vsq[:], 1.0)
                rr2 = sb.tile([1, n], dt.float32, tag="hd_rr2")
                nc.vector.reciprocal(out=rr2[:], in_=vsq[:])
                radv = sb.tile([1, n], dt.float32, tag="hd_radv")
                nc.vector.tensor_scalar(out=radv[:], in0=rr2[:], scalar1=-0.2,
                                        scalar2=1.1, op0=OP.mult, op1=OP.add)
                nc.sync.dma_start(out=rad[:], in_=radv[:])
    nc.finalize()
    return nc


def build_p4():
    nc = bacc.Bacc(None, target_bir_lowering=False)
    ANG = nc.declare_dram_parameter("ANG", [P, 64], dt.float32, isOutput=False)
    RAD = nc.declare_dram_parameter("RAD", [P, 64], dt.float32, isOutput=False)
    CX = nc.declare_dram_parameter("CX", [P, 64], dt.float32, isOutput=True)
    CY = nc.declare_dram_parameter("CY", [P, 64], dt.float32, isOutput=True)
    with tile.TileContext(nc) as tc:
        with (
            tc.tile_pool(name="consts", bufs=1) as consts,
            tc.tile_pool(name="sb", bufs=1) as sb,
            tc.tile_pool(name="ps", bufs=1, space="PSUM") as ps,
        ):
            ones_col = consts.tile([P, 1], dt.float32)
            nc.gpsimd.memset(ones_col[:], 1.0)
            ones_row = consts.tile([1, P], dt.float32)
            nc.gpsimd.memset(ones_row[:], 1.0)
            half_pi = consts.tile([P, 1], dt.float32)
            nc.gpsimd.memset(half_pi[:], PI / 2.0)
            magic = consts.tile([P, 64], dt.int32)
            nc.gpsimd.memset(magic[:], MAGIC)

            ang_t = sb.tile([P, 64], dt.float32)
            nc.sync.dma_start(out=ang_t[:], in_=ANG[:])
            rad_t = sb.tile([P, 64], dt.float32)
            nc.sync.dma_start(out=rad_t[:], in_=RAD[:])
            absang = sb.tile([P, 64], dt.float32)
            nc.scalar.activation(out=absang[:], in_=ang_t[:], func=AF.Abs)
            cosx = sb.tile([P, 64], dt.float32)
            nc.scalar.activation(out=cosx[:], in_=absang[:], func=AF.Sin,
                                 scale=-1.0, bias=half_pi[:])
            sinx = sb.tile([P, 64], dt.float32)
            nc.scalar.activation(out=sinx[:], in_=ang_t[:], func=AF.Sin)
            cx = sb.tile([P, 64], dt.float32)
            nc.vector.tensor_tensor(out=cx[:], in0=rad_t[:], in1=cosx[:], op=OP.mult)
            cy = sb.tile([P, 64], dt.float32)
            nc.vector.tensor_tensor(out=cy[:], in0=rad_t[:], in1=sinx[:], op=OP.mult)
            colsum = sb.tile([P, 2], dt.float32)
            nc.vector.tensor_reduce(out=colsum[:, 0:1], in_=cx[:],
                                    axis=mybir.AxisListType.X, op=OP.add)
            nc.vector.tensor_reduce(out=colsum[:, 1:2], in_=cy[:],
                                    axis=mybir.AxisListType.X, op=OP.add)
            tot_ps = ps.tile([1, 2], dt.float32, space="PSUM")
            nc.tensor.matmul(out=tot_ps[0:1, :], lhsT=ones_col[:], rhs=colsum[:],
                             start=True, stop=True)
            mean = sb.tile([1, 2], dt.float32)
            nc.vector.tensor_scalar_mul(mean[:], tot_ps[0:1, :], 1.0 / N)
            mean_rep = ps.tile([P, 2], dt.float32, space="PSUM")
            nc.tensor.matmul(out=mean_rep[:], lhsT=ones_row[:], rhs=mean[:],
                             start=True, stop=True)
            mrep_sb = sb.tile([P, 2], dt.float32)
            nc.vector.tensor_copy(out=mrep_sb[:], in_=mean_rep[:])
            nc.vector.tensor_tensor(out=cx[:], in0=cx[:],
                                    in1=mrep_sb[:, 0:1].to_broadcast([P, 64]),
                                    op=OP.subtract)
            nc.vector.tensor_tensor(out=cy[:], in0=cy[:],
                                    in1=mrep_sb[:, 1:2].to_broadcast([P, 64]),
                                    op=OP.subtract)
            q = sb.tile([P, 64], dt.float32)
            nc.vector.tensor_tensor(out=q[:], in0=cx[:], in1=cx[:], op=OP.mult)
            cy2 = sb.tile([P, 64], dt.float32)
            nc.vector.tensor_tensor(out=cy2[:], in0=cy[:], in1=cy[:], op=OP.mult)
            nc.vector.tensor_tensor(out=q[:], in0=q[:], in1=cy2[:], op=OP.add)
            nc.vector.tensor_scalar_max(q[:], q[:], 1e-24)
            # rsqrt: DVE bit-trick seed + 2 Newton steps
            iq = sb.tile([P, 64], dt.int32)
            nc.vector.tensor_scalar(out=iq[:], in0=q[:].bitcast(dt.int32), scalar1=1,
                                    scalar2=None, op0=OP.logical_shift_right)
            y = sb.tile([P, 64], dt.float32)
            nc.vector.tensor_tensor(out=y[:].bitcast(dt.int32), in0=magic[:],
                                    in1=iq[:], op=OP.subtract)
            u = sb.tile([P, 64], dt.float32)
            for _ in range(2):
                nc.vector.tensor_tensor(out=u[:], in0=y[:], in1=y[:], op=OP.mult)
                nc.vector.tensor_tensor(out=u[:], in0=u[:], in1=q[:], op=OP.mult)
                nc.vector.tensor_scalar(out=u[:], in0=u[:], scalar1=-0.5, scalar2=1.5,
                                        op0=OP.mult, op1=OP.add)
                nc.vector.tensor_tensor(out=y[:], in0=y[:], in1=u[:], op=OP.mult)
            nc.vector.tensor_tensor(out=cx[:], in0=cx[:], in1=y[:], op=OP.mult)
            nc.vector.tensor_tensor(out=cy[:], in0=cy[:], in1=y[:], op=OP.mult)
            nc.sync.dma_start(out=CX[:], in_=cx[:])
            nc.sync.dma_start(out=CY[:], in_=cy[:])
    nc.finalize()
    return nc


# ----------------------------------------------------------------------------
# orchestration
# ----------------------------------------------------------------------------

_REP16 = np.zeros((8, P), np.float32)
for _h in range(8):
    _REP16[_h, _h * 16 : (_h + 1) * 16] = 1.0


def _launch(prog, in_maps, cores, label=""):
    """Run one program on HW; optionally also CoreSim core-0 for a timing
    estimate (GAT_SIMT=1). Appends time (sim if available, else HW) to
    kernel._last_times."""
    from concourse.bass_utils import run_bass_kernel_spmd

    r = run_bass_kernel_spmd(prog, in_maps, cores)
    t = r.exec_time_ns
    if os.environ.get("GAT_SIMT"):
        import time as _time

        from concourse.bass_interp import CoreSim

        sim = CoreSim(prog, require_finite=False, require_nnan=False)
        for k, v in in_maps[0].items():
            sim.tensor(k)[:] = v
        w0 = _time.time()
        sim.simulate()
        t = sim.time
        print(f"  [simt] {label}: {t} ns (sim wall {_time.time()-w0:.1f}s)")
    kernel._last_times.append(t)
    return r


def kernel(**inputs):
    kernel._last_times = []
    x = np.asarray(inputs["x"], np.float32)
    prep = host_prep(inputs["src"], inputs["dst"])
    order, K = prep["order"], prep["K"]
    cores = list(range(NCORES))

    xT = np.zeros((INP, N), np.float16)
    xT[:IN] = x[order].T.astype(np.float16)
    W1p = np.zeros((INP, HC), np.float32)
    W1p[:IN] = np.asarray(inputs["W1"], np.float32)
    W1x = np.ascontiguousarray(
        W1p.reshape(65, P, HC).transpose(1, 0, 2).reshape(P, 65 * P)
    ).astype(np.float16)
    Mb = {l: mboth(np.asarray(inputs[f"as{l}"], np.float32),
                   np.asarray(inputs[f"ad{l}"], np.float32)).astype(np.float16)
          for l in (1, 2, 3)}
    cols = [core_cols(c) for c in cores]
    rep16 = _REP16.astype(np.float16)

    def mk_tfull(touts):
        Tf = np.zeros((NT, ROW), np.float16)
        for c in cores:
            Tf[cols[c]] = touts[c]
        Tf[N, 128:136] = GHOST_SA
        return Tf

    # ---- P1 ----
    p1 = build_p1()
    in_maps = [dict(xT=np.ascontiguousarray(xT[:, cols[c]]), W1x=W1x, Mb=Mb[1])
               for c in cores]
    r1 = _launch(p1, in_maps, cores, "P1")
    Tfull = mk_tfull([r1.results[c]["Tout"] for c in cores])

    # ---- P2 (layers 2, 3) ----
    p2 = build_p23(K, with_next=True, with_head=False)
    xprev = [np.zeros((P, 1024), np.float32) for _ in cores]
    for l in (2, 3):
        in_maps = []
        for c in cores:
            in_maps.append(dict(
                Tfull=Tfull, Town=np.ascontiguousarray(Tfull[cols[c]]),
                xprev=xprev[c], idxq=prep["idxq"][c],
                bprev=np.asarray(inputs[f"b{l-1}"], np.float32).reshape(P, 1),
                gam=np.asarray(inputs[f"g{l-1}"], np.float32).reshape(P, 1),
                bet=np.asarray(inputs[f"be{l-1}"], np.float32).reshape(P, 1),
                Wn=np.ascontiguousarray(np.asarray(inputs[f"W{l}"], np.float32)).astype(np.float16),
                Mb=Mb[l], rep16q=rep16,
            ))
        r2 = _launch(p2, in_maps, cores, f"P2.l{l}")
        Tfull = mk_tfull([r2.results[c]["Tout"] for c in cores])
        for c in cores:
            xprev[c] = r2.results[c]["xnout"]

    # ---- P3 (layer-3 aggregation + MLP head) ----
    p3 = build_p23(K, with_next=False, with_head=True)
    in_maps = []
    for c in cores:
        in_maps.append(dict(
            Tfull=Tfull, Town=np.ascontiguousarray(Tfull[cols[c]]),
            xprev=xprev[c], idxq=prep["idxq"][c],
            bprev=np.asarray(inputs["b3"], np.float32).reshape(P, 1),
            gam=np.asarray(inputs["g3"], np.float32).reshape(P, 1),
            bet=np.asarray(inputs["be3"], np.float32).reshape(P, 1),
            rep16q=rep16,
            aW1=np.ascontiguousarray(np.asarray(inputs["aW1"], np.float16)),
            ab1=np.asarray(inputs["ab1"], np.float32).reshape(P, 1),
            agm=np.asarray(inputs["ag"], np.float32).reshape(P, 1),
            abe=np.asarray(inputs["abe"], np.float32).reshape(P, 1),
            aW2=np.asarray(inputs["aW2"], np.float16).reshape(P, 1),
            ab2=np.asarray(inputs["ab2"], np.float32).reshape(1, 1),
            rW1=np.ascontiguousarray(np.asarray(inputs["rW1"], np.float16)),
            rb1=np.asarray(inputs["rb1"], np.float32).reshape(64, 1),
            rgm=np.asarray(inputs["rg"], np.float32).reshape(64, 1),
            rbe=np.asarray(inputs["rbe"], np.float32).reshape(64, 1),
            rW2=np.asarray(inputs["rW2"], np.float16).reshape(64, 1),
            rb2=np.asarray(inputs["rb2"], np.float32).reshape(1, 1),
        ))
    r3 = _launch(p3, in_maps, cores, "P3")
    ang = np.zeros(N, np.float32)
    rad = np.zeros(N, np.float32)
    for c in cores:
        ang[cols[c]] = r3.results[c]["ang"][0]
        rad[cols[c]] = r3.results[c]["rad"][0]

    # ---- P4 (finalize, replicated) ----
    p4 = build_p4()
    r4 = _launch(
        p4, [dict(ANG=ang.reshape(P, 64), RAD=rad.reshape(P, 64))] * NCORES, cores,
        "P4")
    cxv = r4.results[0]["CX"].reshape(N)
    cyv = r4.results[0]["CY"].reshape(N)

    out = np.zeros((N, 2), np.float32)
    out[order, 0] = cxv
    out[order, 1] = cyv
    return out


# revision 21
# speedup vs baseline: 1.4067x; 1.2545x over previous
"""Trainium2 Bass kernel for nn_GAT_86045374808682 (3-layer GAT + coordinate head).

Self-contained: takes FULL inputs, shards across 8 NeuronCores internally,
returns the FULL [8192, 2] float32 output.

v2 optimizations over baseline:
- fp16 512-B table rows [h(128)|sa(8)|da(8)|pad]; ghost-node padding (no masks).
- Interleaved feature order f' = e*8 + h (host-side permutation of all params)
  so the per-head edge-weight multiply broadcasts on a middle dim -> DVE 2x.
- One gather per stripe (Pool gen amortized); whole-stripe z/ex/w ops.
- Transpose-accumulate via regular matmul w/ identity rhs (fp16 inputs, exact
  fp32 PSUM accumulation; 16-bit is_transpose accumulation is broken on HW).
- Batched tail: LN/residual/next-table built once on [128, 1024].
- No Ln activations anywhere: rsqrt = DVE bit-trick + 2 Newton steps;
  tanh(softplus(x)) = 1 - 2/((1+e^x)^2+1). Single ACT table set.
- P1: host-swizzled x + W1 -> 4 big DMAs; fp16 matmuls.
- idx lists shipped pre-replicated -> one DMA.
"""
import os
import sys

import numpy as np

for _p in ("/opt/trn_rl_repo", "/root/.axon_site/_ro/trn_rl_repo"):
    if _p not in sys.path:
        sys.path.append(_p)

import concourse.bass as bass  # noqa: F401
import concourse.tile as tile
from concourse import bacc, library_config, mybir
from concourse.masks import make_identity

dt = mybir.dt
AF = mybir.ActivationFunctionType
OP = mybir.AluOpType

N = 8192
IN = 8193
INP = 8320  # 65 * 128
H = 8
HC = 128
P = 128
NCORES = 8
NSTRIPE = 8
ROW = 256  # fp16 elems per table row (512 B)
NT = N + 2  # table rows (ghost at N)
GHOST_SA = -30000.0
PI = float(np.pi)
MAGIC = 0x5F3759DF

# interleaved feature order: new position e*8+h holds old feature h*16+e
IDXP = (np.arange(128) % 8) * 16 + np.arange(128) // 8


# ----------------------------------------------------------------------------
# host-side graph prep
# ----------------------------------------------------------------------------

def host_prep(src, dst):
    s = np.concatenate([np.asarray(src).astype(np.int64), np.arange(N, dtype=np.int64)])
    d = np.concatenate([np.asarray(dst).astype(np.int64), np.arange(N, dtype=np.int64)])
    deg = np.bincount(d, minlength=N)
    order = np.argsort(-deg, kind="stable")  # new-id -> old-id
    old2new = np.empty(N, np.int64)
    old2new[order] = np.arange(N)
    s_new = old2new[s]
    d_new = old2new[d]
    deg_new = deg[order]

    K = [int(deg_new[1024 * t]) for t in range(NSTRIPE)]  # desc-sorted -> stripe max
    offs = np.cumsum([0] + K)

    eo = np.argsort(d_new, kind="stable")
    s_sorted = s_new[eo]
    starts = np.searchsorted(d_new[eo], np.arange(N))

    idxq = np.zeros((NCORES, 16, int(offs[-1]) * 8), np.int16)
    ar = np.arange(P)
    for c in range(NCORES):
        for t in range(NSTRIPE):
            Kt = K[t]
            vids = (t * NCORES + c) * P + ar
            e0 = starts[vids]
            degs = deg_new[vids]
            kk = np.arange(Kt)
            take = np.minimum(e0[:, None] + kk[None, :], len(s_sorted) - 1)
            mat = s_sorted[take]                      # [128, Kt]
            valid = kk[None, :] < degs[:, None]
            mat = np.where(valid, mat, N)             # ghost node for padding
            lin = mat.T.reshape(-1)                   # slot-major [Kt*128]
            o16 = int(offs[t]) * 8
            idxq[c, :, o16 : o16 + Kt * 8] = lin.reshape(-1, 16).T
    idxrep = np.ascontiguousarray(np.tile(idxq, (1, 8, 1)))  # [NCORES, 128, SK*8]
    return dict(order=order, K=K, offs=offs, idxrep=idxrep)


def core_cols(c):
    return np.concatenate([np.arange((t * NCORES + c) * P, (t * NCORES + c) * P + P)
                           for t in range(NSTRIPE)])


def mboth(a_src, a_dst):
    M = np.zeros((P, 16), np.float32)
    for h in range(H):
        M[h * 16 : (h + 1) * 16, h] = a_src[h]
        M[h * 16 : (h + 1) * 16, 8 + h] = a_dst[h]
    return M[IDXP]


# ----------------------------------------------------------------------------
# shared bass building blocks
# ----------------------------------------------------------------------------

def _mk_consts(nc, consts):
    c = {"pool": consts}
    nc.gpsimd.load_library(library_config.mlp)
    c["ident16"] = consts.tile([P, P], dt.float16, name="c_ident16")
    make_identity(nc, c["ident16"][:])
    c["ones_col"] = consts.tile([P, 1], dt.float16, name="c_ones_col")
    nc.gpsimd.memset(c["ones_col"][:], 1.0)
    c["ones_row"] = consts.tile([1, P], dt.float16, name="c_ones_row")
    nc.gpsimd.memset(c["ones_row"][:], 1.0)
    c["magic"] = consts.tile([1, 512], dt.int32, name="c_magic")
    nc.gpsimd.memset(c["magic"][:], MAGIC)
    return c


def _rstd_dve(nc, sb, c, var_ap, out_ap, n, eps):
    """out = 1/sqrt(var+eps), pure DVE: bit-trick seed + 2 Newton steps."""
    vpe = sb.tile([1, 512], dt.float32, tag="rs_vpe")
    if eps:
        nc.vector.tensor_scalar_add(vpe[:, 0:n], var_ap, float(eps))
    else:
        nc.vector.tensor_copy(out=vpe[:, 0:n], in_=var_ap)
    v = vpe[:, 0:n]
    iv = sb.tile([1, 512], dt.int32, tag="rs_iv")
    nc.vector.tensor_scalar(out=iv[:, 0:n], in0=v.bitcast(dt.int32), scalar1=1,
                            scalar2=None, op0=OP.logical_shift_right)
    y = sb.tile([1, 512], dt.float32, tag="rs_y")
    nc.vector.tensor_tensor(out=y[:, 0:n].bitcast(dt.int32), in0=c["magic"][:, 0:n],
                            in1=iv[:, 0:n], op=OP.subtract)
    u = sb.tile([1, 512], dt.float32, tag="rs_u")
    for _ in range(2):
        nc.vector.tensor_tensor(out=u[:, 0:n], in0=y[:, 0:n], in1=y[:, 0:n], op=OP.mult)
        nc.vector.tensor_tensor(out=u[:, 0:n], in0=u[:, 0:n], in1=v, op=OP.mult)
        nc.vector.tensor_scalar(out=u[:, 0:n], in0=u[:, 0:n], scalar1=-0.5, scalar2=1.5,
                                op0=OP.mult, op1=OP.add)
        nc.vector.tensor_tensor(out=y[:, 0:n], in0=y[:, 0:n], in1=u[:, 0:n], op=OP.mult)
    nc.vector.tensor_copy(out=out_ap, in_=y[:, 0:n])


def _ln_relu_fm(nc, sb, ps, c, x_sb, n, gamma_t, beta_t, out_sb, nfeat=P):
    """Feature-major LN + affine + ReLU: out = relu(gamma*(x-mu)*rstd + beta).
    x_sb [nfeat, n] fp32 SBUF; per-column stats; 512-col chunks. Matmul inputs
    go through fp16 copies (1 cyc/row on PE; stats still accumulate fp32)."""
    for j in range(0, n, 512):
        w = min(512, n - j)
        xs = x_sb[:, j : j + w]
        x16 = sb.tile([nfeat, 512], dt.float16, tag="ln_x16")
        nc.vector.tensor_copy(out=x16[:, 0:w], in_=xs)
        xsq = sb.tile([nfeat, 512], dt.float16, tag="ln_xsq")
        nc.scalar.activation(out=xsq[:, 0:w], in_=x16[:, 0:w], func=AF.Square)
        s1_ps = ps.tile([1, 512], dt.float32, space="PSUM", tag="pp_a")
        nc.tensor.matmul(out=s1_ps[:, 0:w], lhsT=c["ones_col"][0:nfeat, :],
                         rhs=x16[:, 0:w], start=True, stop=True)
        s2_ps = ps.tile([1, 512], dt.float32, space="PSUM", tag="pp_b")
        nc.tensor.matmul(out=s2_ps[:, 0:w], lhsT=c["ones_col"][0:nfeat, :],
                         rhs=xsq[:, 0:w], start=True, stop=True)
        mu = sb.tile([1, 512], dt.float32, tag="ln_mu")
        nc.vector.tensor_scalar_mul(mu[:, 0:w], s1_ps[:, 0:w], 1.0 / nfeat)
        musq = sb.tile([1, 512], dt.float32, tag="ln_musq")
        nc.vector.tensor_tensor(out=musq[:, 0:w], in0=mu[:, 0:w], in1=mu[:, 0:w],
                                op=OP.mult)
        var = sb.tile([1, 512], dt.float32, tag="ln_var")
        nc.vector.scalar_tensor_tensor(out=var[:, 0:w], in0=s2_ps[:, 0:w],
                                       scalar=1.0 / nfeat, in1=musq[:, 0:w],
                                       op0=OP.mult, op1=OP.subtract)
        rs = sb.tile([1, 512], dt.float32, tag="ln_rs")
        _rstd_dve(nc, sb, c, var[:, 0:w], rs[:, 0:w], w, 1e-5)
        mu16 = sb.tile([1, 512], dt.float16, tag="ln_mu16")
        nc.vector.tensor_copy(out=mu16[:, 0:w], in_=mu[:, 0:w])
        rs16 = sb.tile([1, 512], dt.float16, tag="ln_rs16")
        nc.vector.tensor_copy(out=rs16[:, 0:w], in_=rs[:, 0:w])
        rep_mu = ps.tile([nfeat, 512], dt.float32, space="PSUM", tag="pp_a")
        nc.tensor.matmul(out=rep_mu[:, 0:w], lhsT=c["ones_row"][:, 0:nfeat],
                         rhs=mu16[:, 0:w], start=True, stop=True)
        rep_rs = ps.tile([nfeat, 512], dt.float32, space="PSUM", tag="pp_b")
        nc.tensor.matmul(out=rep_rs[:, 0:w], lhsT=c["ones_row"][:, 0:nfeat],
                         rhs=rs16[:, 0:w], start=True, stop=True)
        xh = sb.tile([nfeat, 512], dt.float32, tag="ln_xh")
        nc.vector.tensor_tensor(out=xh[:, 0:w], in0=xs, in1=rep_mu[:, 0:w], op=OP.subtract)
        nc.vector.tensor_tensor(out=xh[:, 0:w], in0=xh[:, 0:w], in1=rep_rs[:, 0:w],
                                op=OP.mult)
        nc.scalar.activation(out=out_sb[:, j : j + w], in_=xh[:, 0:w], func=AF.Relu,
                             scale=gamma_t[:], bias=beta_t[:])


# ----------------------------------------------------------------------------
# program builders
# ----------------------------------------------------------------------------

def build_p1():
    nc = bacc.Bacc(None, target_bir_lowering=False)
    xs = nc.declare_dram_parameter("xs", [P, 65 * 1024], dt.float16, isOutput=False)
    W1x = nc.declare_dram_parameter("W1x", [P, 65 * P], dt.float16, isOutput=False)
    Mb = nc.declare_dram_parameter("Mb", [P, 16], dt.float16, isOutput=False)
    Tout = nc.declare_dram_parameter("Tout", [1024, ROW], dt.float16, isOutput=True)

    with tile.TileContext(nc) as tc:
        with (
            tc.tile_pool(name="consts", bufs=1) as consts,
            tc.tile_pool(name="sb", bufs=2) as sb,
            tc.tile_pool(name="psh", bufs=2, space="PSUM") as psh,
            tc.tile_pool(name="ps", bufs=2, space="PSUM") as ps,
        ):
            ident16 = consts.tile([P, P], dt.float16)
            make_identity(nc, ident16[:])
            mb_t = consts.tile([P, 16], dt.float16)
            nc.sync.dma_start(out=mb_t[:], in_=Mb[:])
            w1_t = consts.tile([P, 65, P], dt.float16)
            nc.sync.dma_start(out=w1_t[:], in_=W1x[:].rearrange("p (c j) -> p c j", c=65))
            xs_t = consts.tile([P, 65, 1024], dt.float16)
            xs_v = xs[:].rearrange("p (c j) -> p c j", c=65)
            for a, b in ((0, 22), (22, 44), (44, 65)):
                nc.sync.dma_start(out=xs_t[:, a:b, :], in_=xs_v[:, a:b, :])
            pk = consts.tile([P, ROW], dt.float16)
            nc.vector.memset(pk[:], 0.0)
            for nb in range(2):
                hps = psh.tile([P, 512], dt.float32, space="PSUM", tag="hps")
                for kcb in range(65):
                    nc.tensor.matmul(out=hps[:], lhsT=w1_t[:, kcb, :],
                                     rhs=xs_t[:, kcb, nb * 512 : (nb + 1) * 512],
                                     start=(kcb == 0), stop=(kcb == 64))
                h16 = sb.tile([P, 512], dt.float16, tag="h16")
                nc.vector.tensor_copy(out=h16[:], in_=hps[:])
                sada_ps = ps.tile([16, 512], dt.float32, space="PSUM", tag="pp_a")
                nc.tensor.matmul(out=sada_ps[0:16, :], lhsT=mb_t[:], rhs=h16[:],
                                 start=True, stop=True)
                sada16 = sb.tile([16, 512], dt.float16, tag="sada16")
                nc.vector.tensor_copy(out=sada16[:], in_=sada_ps[0:16, :])
                for b in range(4):
                    blk = nb * 4 + b
                    ht_ps = ps.tile([P, P], dt.float16, space="PSUM", tag="pp_b")
                    nc.tensor.matmul(out=ht_ps[:], lhsT=h16[:, b * P : (b + 1) * P],
                                     rhs=ident16[:], is_transpose=True, start=True,
                                     stop=True)
                    st_ps = ps.tile([P, 16], dt.float16, space="PSUM", tag="pp_c")
                    nc.tensor.matmul(out=st_ps[:], lhsT=sada16[:, b * P : (b + 1) * P],
                                     rhs=ident16[0:16, 0:16], is_transpose=True,
                                     start=True, stop=True)
                    nc.vector.tensor_copy(out=pk[:, 0:128], in_=ht_ps[:])
                    nc.vector.tensor_copy(out=pk[:, 128:144], in_=st_ps[:])
                    nc.sync.dma_start(out=Tout[blk * P : (blk + 1) * P, :], in_=pk[:])
    nc.finalize()
    return nc


def build_p23(K, with_next, with_head):
    """P2 (with_next): edge agg + LN/ReLU/residual + W@ + sada + pack.
    P3 (with_head): edge agg + LN/ReLU/residual + row-norm + MLP head."""
    SK = int(sum(K))
    KMAX = int(max(K))
    offs = np.cumsum([0] + list(K))
    nc = bacc.Bacc(None, target_bir_lowering=False)
    Tfull = nc.declare_dram_parameter("Tfull", [NT, ROW], dt.float16, isOutput=False)
    Town = nc.declare_dram_parameter("Town", [1024, ROW], dt.float16, isOutput=False)
    xprev = nc.declare_dram_parameter("xprev", [P, 1024], dt.float32, isOutput=False)
    idxq = nc.declare_dram_parameter("idxq", [P, SK * 8], dt.int16, isOutput=False)
    bprev = nc.declare_dram_parameter("bprev", [P, 1], dt.float32, isOutput=False)
    gam = nc.declare_dram_parameter("gam", [P, 1], dt.float32, isOutput=False)
    bet = nc.declare_dram_parameter("bet", [P, 1], dt.float32, isOutput=False)
    rep8q = nc.declare_dram_parameter("rep8q", [8, P], dt.float16, isOutput=False)
    if with_next:
        Wn = nc.declare_dram_parameter("Wn", [P, P], dt.float16, isOutput=False)
        Mb = nc.declare_dram_parameter("Mb", [P, 16], dt.float16, isOutput=False)
        Tout = nc.declare_dram_parameter("Tout", [1024, ROW], dt.float16, isOutput=True)
        xnout = nc.declare_dram_parameter("xnout", [P, 1024], dt.float32, isOutput=True)
    if with_head:
        aW1 = nc.declare_dram_parameter("aW1", [P, P], dt.float16, isOutput=False)
        ab1 = nc.declare_dram_parameter("ab1", [P, 1], dt.float32, isOutput=False)
        agm = nc.declare_dram_parameter("agm", [P, 1], dt.float32, isOutput=False)
        abe = nc.declare_dram_parameter("abe", [P, 1], dt.float32, isOutput=False)
        aW2 = nc.declare_dram_parameter("aW2", [P, 1], dt.float16, isOutput=False)
        ab2 = nc.declare_dram_parameter("ab2", [1, 1], dt.float32, isOutput=False)
        rW1 = nc.declare_dram_parameter("rW1", [P, 64], dt.float16, isOutput=False)
        rb1 = nc.declare_dram_parameter("rb1", [64, 1], dt.float32, isOutput=False)
        rgm = nc.declare_dram_parameter("rgm", [64, 1], dt.float32, isOutput=False)
        rbe = nc.declare_dram_parameter("rbe", [64, 1], dt.float32, isOutput=False)
        rW2 = nc.declare_dram_parameter("rW2", [64, 1], dt.float16, isOutput=False)
        rb2 = nc.declare_dram_parameter("rb2", [1, 1], dt.float32, isOutput=False)
        ang = nc.declare_dram_parameter("ang", [1, 1024], dt.float32, isOutput=True)
        rad = nc.declare_dram_parameter("rad", [1, 1024], dt.float32, isOutput=True)

    with tile.TileContext(nc) as tc:
        with (
            tc.tile_pool(name="consts", bufs=1) as consts,
            tc.tile_pool(name="gpool", bufs=2) as gpool,
            tc.tile_pool(name="wpool", bufs=2) as wpool,
            tc.tile_pool(name="sb", bufs=1) as sb,
            tc.tile_pool(name="ps", bufs=2, space="PSUM") as ps,
            tc.tile_pool(name="psagg", bufs=2, space="PSUM") as psagg,
        ):
            c = _mk_consts(nc, consts)
            rep8_t = consts.tile([8, P], dt.float16)
            nc.sync.dma_start(out=rep8_t[:], in_=rep8q[:])

            idx_t = sb.tile([P, SK * 8], dt.int16, tag="idx")
            nc.sync.dma_start(out=idx_t[:], in_=idxq[:])
            da_t = sb.tile([P, NSTRIPE, 8], dt.float16, tag="da")
            nc.sync.dma_start(
                out=da_t[:],
                in_=Town[:].rearrange("(t p) r -> p t r", p=P)[:, :, 136:144])
            xprev_t = sb.tile([P, 1024], dt.float32, tag="xprev")
            nc.sync.dma_start(out=xprev_t[:], in_=xprev[:])
            bias_t = sb.tile([P, 1], dt.float32, tag="bias")
            nc.sync.dma_start(out=bias_t[:], in_=bprev[:])
            gam_t = sb.tile([P, 1], dt.float32, tag="gam")
            nc.sync.dma_start(out=gam_t[:], in_=gam[:])
            bet_t = sb.tile([P, 1], dt.float32, tag="bet")
            nc.sync.dma_start(out=bet_t[:], in_=bet[:])
            if with_next:
                wn_t = sb.tile([P, P], dt.float16, tag="wn")
                nc.sync.dma_start(out=wn_t[:], in_=Wn[:])
                mb_t = sb.tile([P, 16], dt.float16, tag="mb")
                nc.sync.dma_start(out=mb_t[:], in_=Mb[:])

            xmul = sb.tile([P, 1024], dt.float32, tag="xmul")

            # ---- edge phase: one gather per stripe ----
            for t in range(NSTRIPE):
                Kt = int(K[t])
                g = gpool.tile([P, KMAX, ROW], dt.float16, tag="gather")
                nc.gpsimd.dma_gather(
                    out_ap=g[:, 0:Kt, :],
                    in_ap=Tfull[:],
                    idxs_ap=idx_t[:, int(offs[t]) * 8 : int(offs[t + 1]) * 8],
                    num_idxs=Kt * P,
                    num_idxs_reg=Kt * P,
                    elem_size=ROW,
                    single_packet=False,
                )
                z = sb.tile([P, KMAX, 8], dt.float16, tag="z")
                nc.vector.tensor_tensor(
                    out=z[:, 0:Kt, :], in0=g[:, 0:Kt, 128:136],
                    in1=da_t[:, t, :].unsqueeze(1).to_broadcast([P, Kt, 8]),
                    op=OP.add)
                zl = sb.tile([P, KMAX, 8], dt.float16, tag="zl")
                nc.vector.scalar_tensor_tensor(out=zl[:, 0:Kt, :], in0=z[:, 0:Kt, :],
                                               scalar=0.2, in1=z[:, 0:Kt, :],
                                               op0=OP.mult, op1=OP.max)
                ex = sb.tile([P, KMAX, 8], dt.float16, tag="ex")
                nc.scalar.activation(out=ex[:, 0:Kt, :], in_=zl[:, 0:Kt, :], func=AF.Exp)
                den16 = sb.tile([P, 8], dt.float16, tag="den16")
                with nc.allow_low_precision(reason="fp16 den; |den|<=2e3, gate is 2e-2"):
                    nc.vector.tensor_reduce(out=den16[:],
                                            in_=ex[:, 0:Kt, :].transpose([0, 2, 1]),
                                            axis=mybir.AxisListType.X, op=OP.add)
                w = wpool.tile([P, KMAX, P], dt.float16, tag="w")
                nc.vector.tensor_tensor(
                    out=w[:, 0:Kt, :].rearrange("p k (e h) -> p k e h", h=8),
                    in0=g[:, 0:Kt, 0:128].rearrange("p k (e h) -> p k e h", h=8),
                    in1=ex[:, 0:Kt, :].unsqueeze(2).to_broadcast([P, Kt, 16, 8]),
                    op=OP.mult)
                agg = psagg.tile([P, P], dt.float32, space="PSUM", tag="agg")
                for k in range(Kt):
                    # transpose-and-accumulate via regular matmul (w stationary,
                    # identity streaming): 16-bit is_transpose PSUM accumulation
                    # is broken on HW; regular-matmul accumulation is exact.
                    nc.tensor.matmul(out=agg[:], lhsT=w[:, k, :], rhs=c["ident16"][:],
                                     start=(k == 0), stop=(k == Kt - 1))
                dent = ps.tile([8, P], dt.float16, space="PSUM", tag="pp_a")
                nc.tensor.matmul(out=dent[0:8, :], lhsT=den16[:], rhs=c["ident16"][:],
                                 is_transpose=True, start=True, stop=True)
                rden = sb.tile([8, P], dt.float16, tag="rden")
                with nc.allow_low_precision(reason="fp16 1/den; gate is 2e-2"):
                    nc.vector.reciprocal(out=rden[:], in_=dent[0:8, :])
                rdrep = ps.tile([P, P], dt.float32, space="PSUM", tag="pp_b")
                nc.tensor.matmul(out=rdrep[:], lhsT=rep8_t[:], rhs=rden[:],
                                 start=True, stop=True)
                rdrep_sb = sb.tile([P, P], dt.float32, tag="rdrep_sb")
                nc.vector.tensor_copy(out=rdrep_sb[:], in_=rdrep[:])
                nc.vector.tensor_tensor(out=xmul[:, t * P : (t + 1) * P], in0=agg[:],
                                        in1=rdrep_sb[:], op=OP.mult)

            # ---- batched tail: bias + LN + ReLU + residual ----
            xb = sb.tile([P, 1024], dt.float32, tag="xb")
            nc.scalar.activation(out=xb[:], in_=xmul[:], func=AF.Identity,
                                 bias=bias_t[:], scale=1.0)
            xo = sb.tile([P, 1024], dt.float32, tag="xo")
            _ln_relu_fm(nc, sb, ps, c, xb[:], 1024, gam_t, bet_t, xo[:])
            xnext = sb.tile([P, 1024], dt.float32, tag="xnext")
            nc.vector.tensor_tensor(out=xnext[:], in0=xo[:], in1=xprev_t[:], op=OP.add)

            if with_next:
                xs16 = sb.tile([P, 1024], dt.float16, tag="xs16")
                nc.vector.tensor_copy(out=xs16[:], in_=xnext[:])
                hn16 = sb.tile([P, 1024], dt.float16, tag="hn16")
                sada16 = sb.tile([16, 1024], dt.float16, tag="sada16")
                for j in range(0, 1024, 512):
                    hn_ps = ps.tile([P, 512], dt.float32, space="PSUM", tag="pp_a")
                    nc.tensor.matmul(out=hn_ps[:], lhsT=wn_t[:],
                                     rhs=xs16[:, j : j + 512], start=True, stop=True)
                    nc.vector.tensor_copy(out=hn16[:, j : j + 512], in_=hn_ps[:])
                    sada_ps = ps.tile([16, 512], dt.float32, space="PSUM", tag="pp_b")
                    nc.tensor.matmul(out=sada_ps[0:16, :], lhsT=mb_t[:],
                                     rhs=hn16[:, j : j + 512], start=True, stop=True)
                    nc.vector.tensor_copy(out=sada16[:, j : j + 512],
                                          in_=sada_ps[0:16, :])
                pk = sb.tile([P, NSTRIPE, ROW], dt.float16, tag="pk")
                nc.vector.memset(pk[:], 0.0)
                for t in range(NSTRIPE):
                    ht_ps = ps.tile([P, P], dt.float16, space="PSUM", tag="pp_a")
                    nc.tensor.matmul(out=ht_ps[:], lhsT=hn16[:, t * P : (t + 1) * P],
                                     rhs=c["ident16"][:], is_transpose=True,
                                     start=True, stop=True)
                    st_ps = ps.tile([P, 16], dt.float16, space="PSUM", tag="pp_b")
                    nc.tensor.matmul(out=st_ps[:], lhsT=sada16[:, t * P : (t + 1) * P],
                                     rhs=c["ident16"][0:16, 0:16], is_transpose=True,
                                     start=True, stop=True)
                    nc.vector.tensor_copy(out=pk[:, t, 0:128], in_=ht_ps[:])
                    nc.vector.tensor_copy(out=pk[:, t, 128:144], in_=st_ps[:])
                nc.sync.dma_start(out=Tout[:].rearrange("(t p) r -> p t r", p=P),
                                  in_=pk[:])
                nc.sync.dma_start(out=xnout[:], in_=xnext[:])

            if with_head:
                n = 1024
                xn16 = sb.tile([P, n], dt.float16, tag="hd_xn16")
                nc.vector.tensor_copy(out=xn16[:], in_=xnext[:])
                xsq = sb.tile([P, n], dt.float16, tag="hd_xsq")
                nc.scalar.activation(out=xsq[:], in_=xn16[:], func=AF.Square)
                h3n = sb.tile([P, n], dt.float32, tag="hd_h3n")
                for j in range(0, n, 512):
                    ss_ps = ps.tile([1, 512], dt.float32, space="PSUM", tag="pp_a")
                    nc.tensor.matmul(out=ss_ps[0:1, :], lhsT=c["ones_col"][:],
                                     rhs=xsq[:, j : j + 512], start=True, stop=True)
                    ss = sb.tile([1, 512], dt.float32, tag="hd_ss")
                    nc.vector.tensor_scalar_max(ss[:], ss_ps[0:1, :], 1e-24)
                    rn = sb.tile([1, 512], dt.float32, tag="hd_rn")
                    _rstd_dve(nc, sb, c, ss[:], rn[:], 512, 0)
                    rn16 = sb.tile([1, 512], dt.float16, tag="hd_rn16")
                    nc.vector.tensor_copy(out=rn16[:], in_=rn[:])
                    rn_rep = ps.tile([P, 512], dt.float32, space="PSUM", tag="pp_b")
                    nc.tensor.matmul(out=rn_rep[:], lhsT=c["ones_row"][:],
                                     rhs=rn16[:], start=True, stop=True)
                    nc.vector.tensor_tensor(out=h3n[:, j : j + 512], in0=xnext[:, j : j + 512],
                                            in1=rn_rep[:], op=OP.mult)
                h3n16 = sb.tile([P, n], dt.float16, tag="hd_h3n16")
                nc.vector.tensor_copy(out=h3n16[:], in_=h3n[:])

                def mm_bias_act(lhsT_t, rhs16, m, bias_ap, out_sb):
                    for j in range(0, n, 512):
                        mm_ps = ps.tile([P, 512], dt.float32, space="PSUM", tag="pp_a")
                        nc.tensor.matmul(out=mm_ps[0:m, :], lhsT=lhsT_t,
                                         rhs=rhs16[:, j : j + 512], start=True, stop=True)
                        nc.scalar.activation(out=out_sb[:, j : j + 512], in_=mm_ps[0:m, :],
                                             func=AF.Identity, bias=bias_ap, scale=1.0)

                aW1_t = sb.tile([P, P], dt.float16, tag="hd_aW1")
                nc.sync.dma_start(out=aW1_t[:], in_=aW1[:])
                ab1_t = sb.tile([P, 1], dt.float32, tag="hd_ab1")
                nc.sync.dma_start(out=ab1_t[:], in_=ab1[:])
                agm_t = sb.tile([P, 1], dt.float32, tag="hd_agm")
                nc.sync.dma_start(out=agm_t[:], in_=agm[:])
                abe_t = sb.tile([P, 1], dt.float32, tag="hd_abe")
                nc.sync.dma_start(out=abe_t[:], in_=abe[:])
                a_pre = sb.tile([P, n], dt.float32, tag="hd_apre")
                mm_bias_act(aW1_t[:], h3n16, P, ab1_t[:], a_pre)
                a_hid = sb.tile([P, n], dt.float32, tag="hd_ahid")
                _ln_relu_fm(nc, sb, ps, c, a_pre[:], n, agm_t, abe_t, a_hid[:])
                a_hid16 = sb.tile([P, n], dt.float16, tag="hd_ahid16")
                nc.vector.tensor_copy(out=a_hid16[:], in_=a_hid[:])

                aW2_t = sb.tile([P, 1], dt.float16, tag="hd_aW2")
                nc.sync.dma_start(out=aW2_t[:], in_=aW2[:])
                ab2_t = sb.tile([1, 1], dt.float32, tag="hd_ab2")
                nc.sync.dma_start(out=ab2_t[:], in_=ab2[:])
                av = sb.tile([1, n], dt.float32, tag="hd_av")
                mm_bias_act(aW2_t[:], a_hid16, 1, ab2_t[:], av)
                # angles = pi * tanh(av)
                angv = sb.tile([1, n], dt.float32, tag="hd_angv")
                nc.scalar.activation(out=angv[:], in_=av[:], func=AF.Tanh)
                nc.vector.tensor_scalar_mul(angv[:], angv[:], PI)
                nc.sync.dma_start(out=ang[:], in_=angv[:])

                rW1_t = sb.tile([P, 64], dt.float16, tag="hd_rW1")
                nc.sync.dma_start(out=rW1_t[:], in_=rW1[:])
                rb1_t = sb.tile([64, 1], dt.float32, tag="hd_rb1")
                nc.sync.dma_start(out=rb1_t[:], in_=rb1[:])
                rgm_t = sb.tile([64, 1], dt.float32, tag="hd_rgm")
                nc.sync.dma_start(out=rgm_t[:], in_=rgm[:])
                rbe_t = sb.tile([64, 1], dt.float32, tag="hd_rbe")
                nc.sync.dma_start(out=rbe_t[:], in_=rbe[:])
                r_pre = sb.tile([64, n], dt.float32, tag="hd_rpre")
                mm_bias_act(rW1_t[:], h3n16, 64, rb1_t[:], r_pre)
                r_hid = sb.tile([64, n], dt.float32, tag="hd_rhid")
                _ln_relu_fm(nc, sb, ps, c, r_pre[:], n, rgm_t, rbe_t, r_hid[:], nfeat=64)
                r_hid16 = sb.tile([64, n], dt.float16, tag="hd_rhid16")
                nc.vector.tensor_copy(out=r_hid16[:], in_=r_hid[:])

                rW2_t = sb.tile([64, 1], dt.float16, tag="hd_rW2")
                nc.sync.dma_start(out=rW2_t[:], in_=rW2[:])
                rb2_t = sb.tile([1, 1], dt.float32, tag="hd_rb2")
                nc.sync.dma_start(out=rb2_t[:], in_=rb2[:])
                rv = sb.tile([1, n], dt.float32, tag="hd_rv")
                for j in range(0, n, 512):
                    mm_ps = ps.tile([1, 512], dt.float32, space="PSUM", tag="pp_a")
                    nc.tensor.matmul(out=mm_ps[0:1, :], lhsT=rW2_t[:],
                                     rhs=r_hid16[:, j : j + 512], start=True, stop=True)
                    nc.scalar.activation(out=rv[:, j : j + 512], in_=mm_ps[0:1, :],
                                         func=AF.Identity, bias=rb2_t[:], scale=1.0)
                # radius = 1 + 0.1*tanh(softplus(rv)) = 1.1 - 0.2/((1+e^rv)^2 + 1)
                e1 = sb.tile([1, n], dt.float32, tag="hd_e1")
                nc.scalar.activation(out=e1[:], in_=rv[:], func=AF.Exp)
                nc.vector.tensor_scalar_add(e1[:], e1[:], 1.0)
                vsq = sb.tile([1, n], dt.float32, tag="hd_vsq")
                nc.vector.tensor_tensor(out=vsq[:], in0=e1[:], in1=e1[:], op=OP.mult)
                nc.vector.tensor_scalar_add(vsq[:], vsq[:], 1.0)
                rr2 = sb.tile([1, n], dt.float32, tag="hd_rr2")
                nc.vector.reciprocal(out=rr2[:], in_=vsq[:])
                radv = sb.tile([1, n], dt.float32, tag="hd_radv")
                nc.vector.tensor_scalar(out=radv[:], in0=rr2[:], scalar1=-0.2,
                                        scalar2=1.1, op0=OP.mult, op1=OP.add)
                nc.sync.dma_start(out=rad[:], in_=radv[:])
    nc.finalize()
    return nc


def build_p4():
    nc = bacc.Bacc(None, target_bir_lowering=False)
    ANG = nc.declare_dram_parameter("ANG", [P, 64], dt.float32, isOutput=False)
    RAD = nc.declare_dram_parameter("RAD", [P, 64], dt.float32, isOutput=False)
    CX = nc.declare_dram_parameter("CX", [P, 64], dt.float32, isOutput=True)
    CY = nc.declare_dram_parameter("CY", [P, 64], dt.float32, isOutput=True)
    with tile.TileContext(nc) as tc:
        with (
            tc.tile_pool(name="consts", bufs=1) as consts,
            tc.tile_pool(name="sb", bufs=1) as sb,
            tc.tile_pool(name="ps", bufs=1, space="PSUM") as ps,
        ):
            ones_col = consts.tile([P, 1], dt.float32)
            nc.gpsimd.memset(ones_col[:], 1.0)
            ones_row = consts.tile([1, P], dt.float32)
            nc.gpsimd.memset(ones_row[:], 1.0)
            half_pi = consts.tile([P, 1], dt.float32)
            nc.gpsimd.memset(half_pi[:], PI / 2.0)
            magic = consts.tile([P, 64], dt.int32)
            nc.gpsimd.memset(magic[:], MAGIC)

            ang_t = sb.tile([P, 64], dt.float32)
            nc.sync.dma_start(out=ang_t[:], in_=ANG[:])
            rad_t = sb.tile([P, 64], dt.float32)
            nc.sync.dma_start(out=rad_t[:], in_=RAD[:])
            absang = sb.tile([P, 64], dt.float32)
            nc.scalar.activation(out=absang[:], in_=ang_t[:], func=AF.Abs)
            cosx = sb.tile([P, 64], dt.float32)
            nc.scalar.activation(out=cosx[:], in_=absang[:], func=AF.Sin,
                                 scale=-1.0, bias=half_pi[:])
            sinx = sb.tile([P, 64], dt.float32)
            nc.scalar.activation(out=sinx[:], in_=ang_t[:], func=AF.Sin)
            cx = sb.tile([P, 64], dt.float32)
            nc.vector.tensor_tensor(out=cx[:], in0=rad_t[:], in1=cosx[:], op=OP.mult)
            cy = sb.tile([P, 64], dt.float32)
            nc.vector.tensor_tensor(out=cy[:], in0=rad_t[:], in1=sinx[:], op=OP.mult)
            colsum = sb.tile([P, 2], dt.float32)
            nc.vector.tensor_reduce(out=colsum[:, 0:1], in_=cx[:],
                                    axis=mybir.AxisListType.X, op=OP.add)
            nc.vector.tensor_reduce(out=colsum[:, 1:2], in_=cy[:],
                                    axis=mybir.AxisListType.X, op=OP.add)
            tot_ps = ps.tile([1, 2], dt.float32, space="PSUM")
            nc.tensor.matmul(out=tot_ps[0:1, :], lhsT=ones_col[:], rhs=colsum[:],
                             start=True, stop=True)
            mean = sb.tile([1, 2], dt.float32)
            nc.vector.tensor_scalar_mul(mean[:], tot_ps[0:1, :], 1.0 / N)
            mean_rep = ps.tile([P, 2], dt.float32, space="PSUM")
            nc.tensor.matmul(out=mean_rep[:], lhsT=ones_row[:], rhs=mean[:],
                             start=True, stop=True)
            mrep_sb = sb.tile([P, 2], dt.float32)
            nc.vector.tensor_copy(out=mrep_sb[:], in_=mean_rep[:])
            nc.vector.tensor_tensor(out=cx[:], in0=cx[:],
                                    in1=mrep_sb[:, 0:1].to_broadcast([P, 64]),
                                    op=OP.subtract)
            nc.vector.tensor_tensor(out=cy[:], in0=cy[:],
                                    in1=mrep_sb[:, 1:2].to_broadcast([P, 64]),
                                    op=OP.subtract)
            q = sb.tile([P, 64], dt.float32)
            nc.vector.tensor_tensor(out=q[:], in0=cx[:], in1=cx[:], op=OP.mult)
            cy2 = sb.tile([P, 64], dt.float32)
            nc.vector.tensor_tensor(out=cy2[:], in0=cy[:], in1=cy[:], op=OP.mult)
            nc.vector.tensor_tensor(out=q[:], in0=q[:], in1=cy2[:], op=OP.add)
            nc.vector.tensor_scalar_max(q[:], q[:], 1e-24)
            # rsqrt: DVE bit-trick seed + 2 Newton steps
            iq = sb.tile([P, 64], dt.int32)
            nc.vector.tensor_scalar(out=iq[:], in0=q[:].bitcast(dt.int32), scalar1=1,
                                    scalar2=None, op0=OP.logical_shift_right)
            y = sb.tile([P, 64], dt.float32)
            nc.vector.tensor_tensor(out=y[:].bitcast(dt.int32), in0=magic[:],
                                    in1=iq[:], op=OP.subtract)
            u = sb.tile([P, 64], dt.float32)
            for _ in range(2):
                nc.vector.tensor_tensor(out=u[:], in0=y[:], in1=y[:], op=OP.mult)
                nc.vector.tensor_tensor(out=u[:], in0=u[:], in1=q[:], op=OP.mult)
                nc.vector.tensor_scalar(out=u[:], in0=u[:], scalar1=-0.5, scalar2=1.5,
                                        op0=OP.mult, op1=OP.add)
                nc.vector.tensor_tensor(out=y[:], in0=y[:], in1=u[:], op=OP.mult)
            nc.vector.tensor_tensor(out=cx[:], in0=cx[:], in1=y[:], op=OP.mult)
            nc.vector.tensor_tensor(out=cy[:], in0=cy[:], in1=y[:], op=OP.mult)
            nc.sync.dma_start(out=CX[:], in_=cx[:])
            nc.sync.dma_start(out=CY[:], in_=cy[:])
    nc.finalize()
    return nc


# ----------------------------------------------------------------------------
# orchestration
# ----------------------------------------------------------------------------

_REP8 = np.zeros((8, P), np.float32)
_REP8[np.arange(P) % 8, np.arange(P)] = 1.0


def _launch(prog, in_maps, cores, label=""):
    """Run one program on HW; optionally also CoreSim core-0 for a timing
    estimate (GAT_SIMT=1). Appends time (sim if available, else HW) to
    kernel._last_times."""
    from concourse.bass_utils import run_bass_kernel_spmd

    r = run_bass_kernel_spmd(prog, in_maps, cores)
    t = r.exec_time_ns
    if os.environ.get("GAT_SIMT"):
        import time as _time

        from concourse.bass_interp import CoreSim

        sim = CoreSim(prog, require_finite=False, require_nnan=False,
                      trace=bool(os.environ.get("GAT_TRACE")))
        for k, v in in_maps[0].items():
            sim.tensor(k)[:] = v
        w0 = _time.time()
        sim.simulate()
        t = sim.time
        print(f"  [simt] {label}: {t} ns (sim wall {_time.time()-w0:.1f}s)")
    kernel._last_times.append(t)
    return r


def kernel(**inputs):
    kernel._last_times = []
    x = np.asarray(inputs["x"], np.float32)
    prep = host_prep(inputs["src"], inputs["dst"])
    order, K = prep["order"], prep["K"]
    cores = list(range(NCORES))

    W1p = np.zeros((INP, HC), np.float32)
    W1p[:IN] = np.asarray(inputs["W1"], np.float32)[:, IDXP]
    W1x = np.ascontiguousarray(
        W1p.reshape(65, P, HC).transpose(1, 0, 2).reshape(P, 65 * P)
    ).astype(np.float16)
    Mb = {l: mboth(np.asarray(inputs[f"as{l}"], np.float32),
                   np.asarray(inputs[f"ad{l}"], np.float32)).astype(np.float16)
          for l in (1, 2, 3)}
    cols = [core_cols(c) for c in cores]
    rep8 = _REP8.astype(np.float16)

    # per-core swizzled x slabs: xs[p, c, j] = x[node j, feat c*128+p]
    xpad = np.zeros((INP, N), np.float16)
    xpad[:IN] = x[order].T.astype(np.float16)
    xs_by_core = []
    for c in cores:
        sl = xpad[:, cols[c]]  # [INP, 1024]
        xs_by_core.append(np.ascontiguousarray(
            sl.reshape(65, P, 1024).transpose(1, 0, 2).reshape(P, 65 * 1024)))

    def vec(name, perm=True):
        v = np.asarray(inputs[name], np.float32).reshape(-1)
        return (v[IDXP] if perm else v).reshape(-1, 1)

    def mk_tfull(touts):
        Tf = np.zeros((NT, ROW), np.float16)
        for c in cores:
            Tf[cols[c]] = touts[c]
        Tf[N, 128:136] = GHOST_SA
        return Tf

    # ---- P1 ----
    p1 = build_p1()
    in_maps = [dict(xs=xs_by_core[c], W1x=W1x, Mb=Mb[1]) for c in cores]
    r1 = _launch(p1, in_maps, cores, "P1")
    Tfull = mk_tfull([r1.results[c]["Tout"] for c in cores])

    # ---- P2 (layers 2, 3) ----
    p2 = build_p23(K, with_next=True, with_head=False)
    xprev = [np.zeros((P, 1024), np.float32) for _ in cores]
    for l in (2, 3):
        Wn = np.asarray(inputs[f"W{l}"], np.float32)[IDXP][:, IDXP]
        in_maps = []
        for c in cores:
            in_maps.append(dict(
                Tfull=Tfull, Town=np.ascontiguousarray(Tfull[cols[c]]),
                xprev=xprev[c], idxq=prep["idxrep"][c],
                bprev=vec(f"b{l-1}"), gam=vec(f"g{l-1}"), bet=vec(f"be{l-1}"),
                Wn=np.ascontiguousarray(Wn).astype(np.float16),
                Mb=Mb[l], rep8q=rep8,
            ))
        r2 = _launch(p2, in_maps, cores, f"P2.l{l}")
        Tfull = mk_tfull([r2.results[c]["Tout"] for c in cores])
        for c in cores:
            xprev[c] = r2.results[c]["xnout"]

    # ---- P3 (layer-3 aggregation + MLP head) ----
    p3 = build_p23(K, with_next=False, with_head=True)
    in_maps = []
    for c in cores:
        in_maps.append(dict(
            Tfull=Tfull, Town=np.ascontiguousarray(Tfull[cols[c]]),
            xprev=xprev[c], idxq=prep["idxrep"][c],
            bprev=vec("b3"), gam=vec("g3"), bet=vec("be3"),
            rep8q=rep8,
            aW1=np.ascontiguousarray(np.asarray(inputs["aW1"], np.float32)[IDXP]).astype(np.float16),
            ab1=vec("ab1", perm=False),
            agm=vec("ag", perm=False),
            abe=vec("abe", perm=False),
            aW2=np.asarray(inputs["aW2"], np.float16).reshape(P, 1),
            ab2=np.asarray(inputs["ab2"], np.float32).reshape(1, 1),
            rW1=np.ascontiguousarray(np.asarray(inputs["rW1"], np.float32)[IDXP]).astype(np.float16),
            rb1=vec("rb1", perm=False),
            rgm=vec("rg", perm=False),
            rbe=vec("rbe", perm=False),
            rW2=np.asarray(inputs["rW2"], np.float16).reshape(64, 1),
            rb2=np.asarray(inputs["rb2"], np.float32).reshape(1, 1),
        ))
    r3 = _launch(p3, in_maps, cores, "P3")
    ang = np.zeros(N, np.float32)
    rad = np.zeros(N, np.float32)
    for c in cores:
        ang[cols[c]] = r3.results[c]["ang"][0]
        rad[cols[c]] = r3.results[c]["rad"][0]

    # ---- P4 (finalize, replicated) ----
    p4 = build_p4()
    r4 = _launch(
        p4, [dict(ANG=ang.reshape(P, 64), RAD=rad.reshape(P, 64))] * NCORES, cores,
        "P4")
    cxv = r4.results[0]["CX"].reshape(N)
    cyv = r4.results[0]["CY"].reshape(N)

    out = np.zeros((N, 2), np.float32)
    out[order, 0] = cxv
    out[order, 1] = cyv
    return out
